# revision 1
# baseline (speedup 1.0000x reference)
"""Trainium2 Bass kernel for nn_AttentionBlock (GroupNorm + windowed MHA + proj + residual).

Contract: kernel(**inputs) takes FULL unsharded inputs (as from reference.setup_inputs())
and returns the FULL output [1, 256, 96, 96] float32.

Sharding: sequence-parallel over query positions across 8 cores. Each core gets a
uniform slice of each of the 3 reference attention windows:
  W0: q[512i   : 512(i+1)]    attends kv[0    : 6144]
  W1: q[4096+512i : ...]      attends kv[2048 : 9216]
  W2: q[8192+128i : ...]      attends kv[6144 : 9216]
All 4 heads computed locally; projection and residual local too. Every core
redundantly computes GroupNorm stats and full-sequence K/V.

v2 dataflow (single sweep, all 4 heads resident):
  - x [256, 9216] f32r loaded once into SBUF; bn_stats per tile as it arrives.
  - GroupNorm folded into the qkv weights: qkv(a*x+b) = (W*diag(a))x + W*b, so
    wT rows are scaled by a in place (one DVE op) and W*b lands in the per-dim
    bias vector; no xn tensor is ever materialized.
  - x shipped from host as bf16 (stats/kv-gen precision ~0.2%; residual and
    q-gen use the separate f32 xq input, so the residual is exact).
  - k/q stored fp8e4 in DoubleRow pair layout [32, 2, n] per head at
    partitions 32h (dim d -> partition d//2, slot d%2), produced by PSUM->fp8
    copy + 2-tile-batched SBUF fold-DMAs on the sync queue. v stored fp8e4
    [128 keys, chunk, head, 72], dims at 0:64 + ones column at 64.
  - QK and PV matmuls run fp8 DoubleRow (2 cols/cycle, contraction 64/256).
  - exp on ScalarE with scale=0.125, bias=-3.5 (shift cancels in softmax;
    keeps exp well inside fp8e4 range even with fp8 q/k noise - at -2 the
    hottest W0 logits saturated/overflowed), writing fp8 pair tiles that PV
    consumes directly. rstd = exp(-0.5*ln(var+eps)) keeps ACT on the one
    Ln/Exp table set (no ~2.7us table swaps).
  - sums via ones column of v; normalize by a rank-1 PE matmul broadcast of
    the reciprocal row (no DRAM bounce). attn slice j holds heads (j, 2+j)
    (projT rows host-permuted); heads 2,3 shift partitions via one DMA.
  - proj in bf16; v-bias (qkv_b_v + W_v b) folded into the output bias via
    pbtot = proj_b + P @ bias_v (runtime rank-1); residual from xq.

GroupNorm stats are estimated from every other seq tile (36864 samples per
group): estimator noise ~0.4% of sigma, raising end-to-end rel err from
2.3e-3 to 3.4e-3 (gate 2e-2) while halving the serial head-phase DVE.

Measured (repeat-slope metric, same harness for both): v1 baseline ~457us,
this version ~452 vs ~462us for full stats; cost-model sim: 393 -> 282us.
HW overlaps the PE->ACT->PE chain worse than the sim models (deletion
probes: exp marginal ~147us, PV ~40us, QK+qkv+head/tail ~248us). PSUM is
fully allocated (sps 2x2 + accps 2 + ops 2 = 8 banks), blocking deeper S
buffering and 4-chunk exp batches.
"""

import numpy as np
import ml_dtypes

import concourse.bass as bass
import concourse.tile as tile
from concourse import mybir
from concourse.vector_clock import ScopedClock, VectorClock

F32 = mybir.dt.float32
F32R = mybir.dt.float32r
BF16 = mybir.dt.bfloat16
FP8 = mybir.dt.float8e4
AF = mybir.ActivationFunctionType
ALU = mybir.AluOpType
DR = mybir.MatmulPerfMode.DoubleRow

C = 256
SEQ = 9216
NCORES = 8
HEADS = 4
D = 64
EPS = 1e-5
SCALE = 0.125  # 1/sqrt(64)
EXPB = -3.5  # exp shift; cancels in softmax, keeps e^(s*SCALE+EXPB) well under 448 (fp8e4) even with fp8 q/k noise
NQC = 1152  # queries per core
ST = 512  # seq tile for qkv streaming
NST = SEQ // ST  # 18
NCH = SEQ // 128  # 72 key chunks
# windows: (q_off in NQC, q_len, key_chunk0, n_key_chunks)
WINDOWS = [(0, 512, 0, 48), (512, 512, 16, 56), (1024, 128, 48, 24)]


def input_shapes():
    """External-input name -> (shape, np dtype) for build_nc()'s signature."""
    return {
        "x": ((C, SEQ), ml_dtypes.bfloat16),
        "xq": ((C, NQC), np.float32),
        "wT": ((C, 3 * C), np.float32),
        "projTb": ((C, C), ml_dtypes.bfloat16),
        "projT32": ((C, C), np.float32),
        "qkvb": ((128, 6), np.float32),
        "nwb": ((128, 4), np.float32),
        "projbrc": ((128, 2), np.float32),
        "G": ((128, 16), np.float32),
        "GT": ((16, 128), np.float32),
    }


def _patch_tile_drain():
    """This container's walrus rejects >1 sem wait on one sync CTRL instruction
    ("Too many sync wait commands"). Split the TileContext-exit drain's waits
    into one-wait-per-nop instructions."""
    if getattr(tile.TileContext, "_drain_split_patched", False):
        return

    def _drain_and_barrier(self, tick_clock, wait_clock):
        vc = tick_clock.global_clock
        n = len(vc)
        for p in range(n):
            t = vc[p]
            if t <= 0:
                continue
            single = VectorClock([t if i == p else 0 for i in range(n)])
            inst = self.nc.sync.nop(nofuse=True, hint="drain_split_wait")
            wait_clock.add_sem_waits(inst.ins, ScopedClock({None: single}))
        self.nc.sync.drain()
        self.nc.all_engine_barrier()
        assert self.sems is not None
        popped = self.nc._tile_sem_poison_stack.pop()
        assert popped is self._sem_poison
        self.nc.clear_and_free_semaphores(list(self.sems.allocated().values()))
        self.nc.all_engine_barrier()

    tile.TileContext._drain_and_barrier = _drain_and_barrier
    tile.TileContext._drain_split_patched = True


def _patch_to_json_split_waits():
    """This walrus build rejects instructions carrying more than one sem-wait.
    Post-process the BIR JSON: keep one wait per instruction, move extras onto
    same-engine NoOps inserted just before it."""
    if getattr(bass.Bass, "_split_waits_patched", False):
        return
    import json as _json

    orig = bass.Bass.to_json_bytes

    def to_json_bytes(self):
        d = _json.loads(orig(self))
        for fn in d["functions"]:
            for blk in fn["blocks"]:
                out = []
                changed = False
                for ins in blk["instructions"]:
                    si = ins.get("sync_info")
                    waits = (si or {}).get("on_wait") or []
                    if len(waits) > 1:
                        changed = True
                        for k, w in enumerate(waits[:-1]):
                            out.append({
                                "debug": ins.get("debug", 0),
                                "engine": ins["engine"],
                                "ins": [],
                                "name": f"{ins['name']}-w{k}",
                                "opcode": "NoOp",
                                "outs": [],
                                "sync_info": {"on_update": [], "on_wait": [w]},
                                "text_hint": "split_wait",
                            })
                        si["on_wait"] = [waits[-1]]
                    out.append(ins)
                if changed:
                    blk["instructions"] = out
        return _json.dumps(d).encode()

    bass.Bass.to_json_bytes = to_json_bytes
    bass.Bass._split_waits_patched = True


def _bcast_free(ap, n):
    """[p, 1] AP -> [p, n, 1] AP broadcasting along a free dim (step 0)."""
    apl = [list(d) for d in ap.ap]
    return bass.AP(tensor=ap.tensor, offset=ap.offset, ap=[apl[0], [0, n]] + apl[1:])


def _bcast_part(ap, n):
    """[1, m] AP -> [n, m] AP broadcasting along partitions (step 0)."""
    apl = ap.ap
    assert apl[0][1] == 1, apl
    return bass.AP(tensor=ap.tensor, offset=ap.offset, ap=[[0, n]] + [list(d) for d in apl[1:]])


PROBE = None  # None | 'noexp' (skip exp+PV) | 'nopv' (skip PV)


def build_nc(repeat=1):
    nc = bass.Bass()

    x_d = nc.dram_tensor("x", [C, SEQ], BF16, kind="ExternalInput")
    xq_d = nc.dram_tensor("xq", [C, NQC], F32R, kind="ExternalInput")
    wT_d = nc.dram_tensor("wT", [C, 3 * C], F32R, kind="ExternalInput")
    projTb_d = nc.dram_tensor("projTb", [C, C], BF16, kind="ExternalInput")
    projT32_d = nc.dram_tensor("projT32", [C, C], F32R, kind="ExternalInput")
    qkvb_d = nc.dram_tensor("qkvb", [128, 6], F32, kind="ExternalInput")
    nwb_d = nc.dram_tensor("nwb", [128, 4], F32, kind="ExternalInput")
    projbrc_d = nc.dram_tensor("projbrc", [128, 2], F32, kind="ExternalInput")
    G_d = nc.dram_tensor("G", [128, 16], F32, kind="ExternalInput")
    GT_d = nc.dram_tensor("GT", [16, 128], F32, kind="ExternalInput")
    out_d = nc.dram_tensor("out", [C, NQC], F32, kind="ExternalOutput")

    with tile.TileContext(nc) as tc:
        for _rep in range(repeat):
            with (
                tc.tile_pool(name="singles", bufs=1) as singles,
                tc.tile_pool(name="stg", bufs=3) as stg,
                tc.tile_pool(name="pt", bufs=3) as ptp,
                tc.tile_pool(name="epi", bufs=3) as epi,
                tc.tile_pool(name="outp", bufs=2) as outp,
                tc.tile_pool(name="pg", bufs=4) as pg,
                tc.tile_pool(name="sps", bufs=2, space="PSUM") as sps,
                tc.tile_pool(name="accps", bufs=2, space="PSUM") as accps,
                tc.tile_pool(name="ops", bufs=2, space="PSUM") as ops,
                tc.tile_pool(name="dr", bufs=2, space="DRAM") as drp,
            ):
                # ---- x resident + streaming stats (loads first: keeps the
                # DMA queues clear of constant traffic) ----
                x_sb = singles.tile([128, 2, SEQ], BF16, tag="x_sb", name="x_sb")
                XC = 2048  # x dma chunk
                for b0 in range(0, SEQ, XC):
                    b1 = min(SEQ, b0 + XC)
                    nc.sync.dma_start(out=x_sb[:, 0, b0:b1], in_=x_d[0:128, b0:b1])
                    nc.gpsimd.dma_start(out=x_sb[:, 1, b0:b1], in_=x_d[128:256, b0:b1])

                # ---- constants ----
                wT_sb = singles.tile([128, 2, 3 * C], F32R, tag="wT")
                nc.sync.dma_start(out=wT_sb[:, 0, :], in_=wT_d[0:128, :])
                nc.sync.dma_start(out=wT_sb[:, 1, :], in_=wT_d[128:256, :])
                xq_sb = singles.tile([128, 2, NQC], F32R, tag="xq")
                nc.sync.dma_start(out=xq_sb[:, 0, :], in_=xq_d[0:128, :])
                nc.sync.dma_start(out=xq_sb[:, 1, :], in_=xq_d[128:256, :])
                G_sb = singles.tile([128, 16], F32, tag="G")
                nc.gpsimd.dma_start(out=G_sb, in_=G_d[:, :])
                GT_sb = singles.tile([16, 128], F32, tag="GT")
                nc.gpsimd.dma_start(out=GT_sb, in_=GT_d[:, :])
                qkvb_sb = singles.tile([128, 6], F32, tag="qkvb")
                nc.gpsimd.dma_start(out=qkvb_sb, in_=qkvb_d[:, :])
                nwb_sb = singles.tile([128, 4], F32, tag="nwb")
                nc.gpsimd.dma_start(out=nwb_sb, in_=nwb_d[:, :])
                projbrc_sb = singles.tile([128, 2], F32, tag="projbrc")
                nc.gpsimd.dma_start(out=projbrc_sb, in_=projbrc_d[:, :])
                projT32_sb = singles.tile([128, 2, C], F32R, tag="projT32")
                nc.gpsimd.dma_start(out=projT32_sb[:, 0, :], in_=projT32_d[0:128, :])
                nc.gpsimd.dma_start(out=projT32_sb[:, 1, :], in_=projT32_d[128:256, :])
                projTb_sb = singles.tile([128, 2, C], BF16, tag="projTb")
                nc.gpsimd.dma_start(out=projTb_sb[:, 0, :], in_=projTb_d[0:128, :])
                nc.gpsimd.dma_start(out=projTb_sb[:, 1, :], in_=projTb_d[128:256, :])

                # streaming stats (DVE, pipelined with the x chunks above).
                # Subsampled: every other tile (36864 samples per group) -
                # mean/rstd estimator noise ~0.4% of sigma, far inside the
                # accuracy budget, and it halves the serial head-phase DVE.
                NSS = NST // 2
                stats = singles.tile([128, 2, NSS, 6], F32, tag="stats")
                for si in range(NSS):
                    s0 = ST * 2 * si
                    for cc in range(2):
                        nc.vector.bn_stats(out=stats[:, cc, si, :], in_=x_sb[:, cc, s0:s0 + ST])

                epsb_sb = singles.tile([128, 1], F32, tag="epsb")
                nc.vector.memset(epsb_sb, EPS)
                warm = pg.tile([1, 1], F32, tag="warm")
                nc.vector.memset(warm, 0.0)
                nc.scalar.activation(out=warm, in_=warm, func=AF.Exp)

                # ---- GroupNorm a,b; fold into weights + biases ----
                ab_sb = singles.tile([128, 2, 2], F32, tag="ab")  # (a, b) per cc
                bvec_r = singles.tile([128, 2, 2], F32R, tag="bvec_r")  # b per cc, 2x dup (f32r)
                for cc in range(2):
                    mv = pg.tile([128, 2], F32, tag="mv")
                    nc.vector.bn_aggr(out=mv, in_=stats[:, cc, :, :])
                    st2 = pg.tile([128, 2], F32, tag="st2")  # (mean, E[x^2])
                    nc.vector.tensor_copy(out=st2[:, 0:1], in_=mv[:, 0:1])
                    nc.vector.tensor_tensor(out=st2[:, 1:2], in0=mv[:, 0:1], in1=mv[:, 0:1], op=ALU.mult)
                    nc.vector.tensor_tensor(out=st2[:, 1:2], in0=st2[:, 1:2], in1=mv[:, 1:2], op=ALU.add)
                    gps = accps.tile([128, 512], F32, tag="acc")
                    nc.tensor.matmul(gps[0:16, 0:2], lhsT=G_sb, rhs=st2, start=True, stop=True)
                    gm = pg.tile([16, 2], F32, tag="gm")  # (mean_g, E2_g)
                    nc.vector.tensor_copy(out=gm, in_=gps[0:16, 0:2])
                    t16 = pg.tile([16, 1], F32, tag="t16")
                    nc.vector.tensor_tensor(out=t16, in0=gm[:, 0:1], in1=gm[:, 0:1], op=ALU.mult)
                    nc.vector.tensor_tensor(out=gm[:, 1:2], in0=gm[:, 1:2], in1=t16, op=ALU.subtract)
                    # rstd = exp(-0.5*ln(var+eps)): stays in the Ln/Exp
                    # ACT table set, so no table swap before the big exps
                    nc.scalar.activation(out=gm[:, 1:2], in_=gm[:, 1:2], func=AF.Ln,
                                         bias=epsb_sb[0:16, :])
                    nc.scalar.activation(out=gm[:, 1:2], in_=gm[:, 1:2], func=AF.Exp,
                                         scale=-0.5)
                    mps = accps.tile([128, 512], F32, tag="acc")
                    nc.tensor.matmul(mps[0:128, 0:2], lhsT=GT_sb, rhs=gm, start=True, stop=True)
                    mr = pg.tile([128, 2], F32, tag="mr")  # (mean_c, rstd_c)
                    nc.vector.tensor_copy(out=mr, in_=mps[0:128, 0:2])
                    # a = rstd * norm_w ; b = norm_b - mean * a
                    nc.vector.tensor_tensor(out=ab_sb[:, cc, 0:1], in0=mr[:, 1:2], in1=nwb_sb[:, cc:cc + 1], op=ALU.mult)
                    t128 = pg.tile([128, 1], F32, tag="t128")
                    nc.vector.tensor_tensor(out=t128, in0=mr[:, 0:1], in1=ab_sb[:, cc, 0:1], op=ALU.mult)
                    nc.vector.tensor_tensor(out=ab_sb[:, cc, 1:2], in0=nwb_sb[:, 2 + cc:3 + cc], in1=t128, op=ALU.subtract)
                    nc.vector.tensor_copy(out=bvec_r[:, cc, :], in_=_bcast_free(ab_sb[:, cc, 1:2], 2))

                # bias_all = qkvb + W @ b  (uses UNSCALED wT; emitted before the
                # in-place scale below, so Tile orders the reads first)
                bps = accps.tile([128, 512], F32, tag="acc", name="bps")
                for s in range(6):
                    for cc in range(2):
                        nc.tensor.matmul(
                            bps[0:128, 2 * s:2 * s + 2], lhsT=wT_sb[:, cc, 128 * s:128 * (s + 1)],
                            rhs=bvec_r[:, cc, :],
                            start=(cc == 0), stop=(cc == 1))
                bias_all = singles.tile([128, 6], F32, tag="bias_all")
                nc.vector.tensor_tensor(out=bias_all, in0=qkvb_sb,
                                        in1=bps[0:128, 0:12].rearrange("p (s j) -> p s j", j=2)[:, :, 0],
                                        op=ALU.add)
                # pbtot = proj_b + P @ bias_v  (folds the v bias through proj)
                bvv = singles.tile([128, 2, 2], F32R, tag="bvv")
                for ds in range(2):
                    nc.vector.tensor_copy(out=bvv[:, ds, :], in_=_bcast_free(bias_all[:, 4 + ds:5 + ds], 2))
                pbps = accps.tile([128, 512], F32, tag="acc", name="pbps")
                for mc in range(2):
                    for ds in range(2):
                        nc.tensor.matmul(
                            pbps[0:128, 2 * mc:2 * mc + 2], lhsT=projT32_sb[:, ds, 128 * mc:128 * (mc + 1)],
                            rhs=bvv[:, ds, :], start=(ds == 0), stop=(ds == 1))
                pbtot = singles.tile([128, 2], F32, tag="pbtot")
                nc.vector.tensor_tensor(out=pbtot, in0=projbrc_sb,
                                        in1=pbps[0:128, 0:4].rearrange("p (s j) -> p s j", j=2)[:, :, 0],
                                        op=ALU.add)
                # scaled weight copies: bf16 (k/v-gen vs bf16 x) and f32r
                # q-slice (q-gen vs f32r xq)
                wTs_b = singles.tile([128, 2, 2 * C], BF16, tag="wTs_b")
                wTs_q = singles.tile([128, 2, C], F32R, tag="wTs_q")
                for cc in range(2):
                    nc.vector.tensor_scalar_mul(
                        out=wTs_b[:, cc, :], in0=wT_sb[:, cc, C:3 * C],
                        scalar1=ab_sb[:, cc, 0:1])
                    nc.vector.tensor_scalar_mul(
                        out=wTs_q[:, cc, :], in0=wT_sb[:, cc, 0:C],
                        scalar1=ab_sb[:, cc, 0:1])

                # ---- fp8 k/v/q caches ----
                # k: [p, slot, key] per head at partitions 32h..32h+32; dim d of
                # head -> partition 32h + d//2, slot d%2
                k_sb = singles.tile([128, 2, SEQ], FP8, tag="k_sb", name="k_sb")
                # v: [key-in-chunk, chunk, head, 72]; dims at 0:64, ones at 64
                v_sb = singles.tile([128, NCH, HEADS, 72], FP8, tag="v_sb", name="v_sb")
                ones_f = singles.tile([128, 1], F32, tag="ones_f")
                nc.vector.memset(ones_f, 1.0)
                ones_v_bc = bass.AP(tensor=ones_f.tensor, offset=ones_f.offset,
                                    ap=[list(ones_f.ap[0]), [0, NCH], [0, HEADS], [1, 1]])
                nc.vector.tensor_copy(out=v_sb[:, :, :, 64:65], in_=ones_v_bc)
                q_sb = singles.tile([128, 2, NQC], FP8, tag="q_sb", name="q_sb")
                expb_sb = singles.tile([128, 1], F32, tag="expb")
                nc.vector.memset(expb_sb, EXPB)
                ones_r = singles.tile([1, 64], F32R, tag="ones_r")
                nc.vector.tensor_copy(out=ones_r, in_=_bcast_free(ones_f[0:1, 0:1], 64))

                # ---- q generation (per window, both dim-slices) ----
                def emit_q(wsel):
                    for s in range(2):  # dim slice: heads (0,1) / (2,3)
                        for w, (q0, qn, _, _) in ((w, WINDOWS[w]) for w in wsel):
                            qps = accps.tile([128, 512], F32, tag="acc", name="qps")
                            for cc in range(2):
                                nc.tensor.matmul(
                                    qps[:, 0:qn],
                                    lhsT=wTs_q[:, cc, 128 * s:128 * s + 128],
                                    rhs=xq_sb[:, cc, q0:q0 + qn], start=(cc == 0), stop=(cc == 1))
                            qstage = stg.tile([128, 512], FP8, tag="stage", name="qstage")
                            nc.vector.tensor_scalar_add(
                                out=qstage[:, 0:qn], in0=qps[:, 0:qn], scalar1=bias_all[:, s:s + 1])
                            for hp in range(2):  # head within slice
                                h = 2 * s + hp
                                eng = nc.sync if hp == 0 else nc.gpsimd
                                eng.dma_start(
                                    out=q_sb[32 * h:32 * h + 32, :, q0:q0 + qn],
                                    in_=qstage[64 * hp:64 * hp + 64, 0:qn])

                # ---- k/v generation for one seq tile ----
                # k staged fp8 per slice in 2-tile groups; one fold-DMA per
                # head per group ([64, 1024] contiguous -> cheap descriptors)
                kstage2 = {}

                def emit_kv_tile(st):
                    s0 = ST * st
                    g = st % 2
                    for s in range(2):  # k dim slice: heads (0,1) / (2,3)
                        kps = accps.tile([128, 512], F32, tag="acc", name="kps")
                        for cc in range(2):
                            nc.tensor.matmul(
                                kps, lhsT=wTs_b[:, cc, 128 * s:128 * s + 128],
                                rhs=x_sb[:, cc, s0:s0 + ST], start=(cc == 0), stop=(cc == 1))
                        if g == 0:
                            kstage2[s] = stg.tile([128, 2, 512], FP8, tag=f"stage{s}", name="kstage2")
                        nc.vector.tensor_scalar_add(
                            out=kstage2[s][:, g, :], in0=kps, scalar1=bias_all[:, 2 + s:3 + s])
                        if g == 1:
                            f0 = ST * (st - 1)
                            for hp in range(2):
                                h = 2 * s + hp
                                nc.sync.dma_start(
                                    out=k_sb[32 * h:32 * h + 32, :, f0:f0 + 2 * ST],
                                    in_=kstage2[s][64 * hp:64 * hp + 64, :, :])
                    # v: 4 token chunks of 128; out [128 tokens, 256 dims]
                    vps = accps.tile([128, 512], F32, tag="acc", name="vps")
                    for mc in range(4):
                        o = 256 * (mc % 2)
                        for cc in range(2):
                            nc.tensor.matmul(
                                vps[:, o:o + 256],
                                lhsT=x_sb[:, cc, s0 + 128 * mc:s0 + 128 * (mc + 1)],
                                rhs=wTs_b[:, cc, C:2 * C], start=(cc == 0), stop=(cc == 1))
                        ch = 4 * st + mc
                        vv = vps[:, o:o + 256].rearrange("p (h d) -> p h d", h=4)
                        nc.vector.tensor_copy(out=v_sb[:, ch, :, 0:64], in_=vv)

                # ---- attention stream for one (window, head) ----
                # attn layout: slice j holds heads (j, 2+j); head h -> slice h%2,
                # partitions 64*(h//2). projTb rows are host-permuted to match.
                def gen_stream(w, h):
                    q0, qn, kc0, nch = WINDOWS[w]
                    o_t = ops.tile([128, 512], F32, tag="o", name="o_t") if not PROBE else None
                    npair = nch // 2
                    for pr in range(npair):
                        c0 = kc0 + 2 * pr
                        s_ps = sps.tile([128, 2, 512], F32, tag="s", name="s_ps")
                        for j in range(2):
                            nc.tensor.matmul(
                                s_ps[:, j, 0:qn],
                                lhsT=k_sb[32 * h:32 * h + 32, :, 128 * (c0 + j):128 * (c0 + j) + 128],
                                rhs=q_sb[32 * h:32 * h + 32, :, q0:q0 + qn],
                                start=True, stop=True, perf_mode=DR,
                                tile_position=(32 * h, 0))
                        if PROBE != 'noexp':
                            pt = ptp.tile([128, 2, 512], FP8, tag="p", name="pt")
                            nc.scalar.activation(out=pt[:, :, 0:qn], in_=s_ps[:, :, 0:qn],
                                                 func=AF.Exp, scale=SCALE, bias=expb_sb)
                            if not PROBE:
                                nc.tensor.matmul(
                                    o_t[0:65, 0:qn],
                                    lhsT=v_sb[:, c0:c0 + 2, h, 0:65],
                                    rhs=pt[:, :, 0:qn],
                                    start=(pr == 0), stop=(pr == npair - 1), perf_mode=DR)
                        yield
                    if PROBE:
                        dbase = 64 * (h // 2)
                        nc.vector.tensor_copy(
                            out=attn_sb[dbase:dbase + 64, h % 2, q0:q0 + qn],
                            in_=_bcast_free(ones_f[0:64, 0:1], qn))
                        return
                    # epilogue: normalize by reciprocal of sums row 64,
                    # partition-broadcast via a rank-1 PE matmul (ones x rec)
                    osb = epi.tile([65, 512], F32, tag="osb", name="osb")
                    nc.vector.tensor_copy(out=osb[:, 0:qn], in_=o_t[0:65, 0:qn])
                    rec = epi.tile([1, 512], F32R, tag="rec", name="rec")
                    with nc.allow_low_precision(reason="f32r out is bit-identical to f32"):
                        nc.vector.reciprocal(out=rec[0:1, 0:qn], in_=osb[64:65, 0:qn])
                    recb = accps.tile([128, 512], F32, tag="acc", name="recb")
                    nc.tensor.matmul(recb[0:64, 0:qn], lhsT=ones_r, rhs=rec[0:1, 0:qn],
                                     start=True, stop=True)
                    if h < 2:
                        nc.vector.tensor_tensor(
                            out=attn_sb[0:64, h, q0:q0 + qn],
                            in0=osb[0:64, 0:qn], in1=recb[0:64, 0:qn], op=ALU.mult)
                    else:
                        at = epi.tile([64, 512], BF16, tag="at", name="at")
                        nc.vector.tensor_tensor(
                            out=at[:, 0:qn], in0=osb[0:64, 0:qn], in1=recb[0:64, 0:qn], op=ALU.mult)
                        nc.sync.dma_start(out=attn_sb[64:128, h - 2, q0:q0 + qn], in_=at[:, 0:qn])

                attn_sb = singles.tile([128, 2, NQC], BF16, tag="attn", name="attn_sb")

                def emit_proj(w):
                    q0, qn, _, _ = WINDOWS[w]
                    for mc in range(2):
                        pp = accps.tile([128, 512], F32, tag="acc", name="pp")
                        for ds in range(2):
                            nc.tensor.matmul(
                                pp[:, 0:qn], lhsT=projTb_sb[:, ds, 128 * mc:128 * (mc + 1)],
                                rhs=attn_sb[:, ds, q0:q0 + qn], start=(ds == 0), stop=(ds == 1))
                        ot = outp.tile([128, 512], F32, tag="ot", name="ot")
                        nc.vector.tensor_scalar_add(out=ot[:, 0:qn], in0=pp[:, 0:qn],
                                                    scalar1=pbtot[:, mc:mc + 1])
                        nc.vector.tensor_tensor(out=ot[:, 0:qn], in0=ot[:, 0:qn],
                                                in1=xq_sb[:, mc, q0:q0 + qn], op=ALU.add)
                        nc.sync.dma_start(out=out_d[128 * mc:128 * (mc + 1), q0:q0 + qn], in_=ot[:, 0:qn])

                # ---- schedule ----
                emit_q([0])

                # head-pair round-robin streams per window, gated on kv tiles.
                # stream (w,h) pair pr needs chunks kc0+2pr+1 <= last emitted
                def ready_pairs(w, last_chunk):
                    _, _, kc0, nch = WINDOWS[w]
                    return min(nch // 2, max(0, (last_chunk - kc0 - 1) // 2 + 1))

                # phase 1: emit kv tiles, interleaving W0 streams for heads 0,1
                # then 2,3; then W1, W2 the same way once their chunks exist.
                gens = {}
                done_pairs = {}
                for w in range(3):
                    for h in range(HEADS):
                        gens[(w, h)] = gen_stream(w, h)
                        done_pairs[(w, h)] = 0

                def advance(w, hpair, upto):
                    """Advance streams (w, 2*hpair) and (w, 2*hpair+1) round-robin
                    until each has emitted `upto` pairs (or exhausted)."""
                    hs = (2 * hpair, 2 * hpair + 1)
                    while True:
                        progressed = False
                        for h in hs:
                            if done_pairs[(w, h)] < upto:
                                try:
                                    next(gens[(w, h)])
                                    done_pairs[(w, h)] += 1
                                    progressed = True
                                except StopIteration:
                                    done_pairs[(w, h)] = 10 ** 9
                        if not progressed:
                            break

                # +1: the final next() runs the epilogue (StopIteration)
                NP = [WINDOWS[w][3] // 2 + 1 for w in range(3)]
                for st in range(NST):
                    emit_kv_tile(st)
                    if st == 2:
                        emit_q([1])
                    elif st == 4:
                        emit_q([2])
                    if st % 2 == 0:
                        continue  # chunks land with the fold at odd st
                    last_chunk = 4 * st + 3
                    # W0 heads 0,1 track the kv frontier; heads 2,3 fill the
                    # tail tiles (W0 chunks all ready from tile 11)
                    advance(0, 0, ready_pairs(0, last_chunk))
                    if st >= 11:
                        advance(0, 0, NP[0])  # epilogues before heads 2,3 start
                    if st >= 13:
                        advance(0, 1, 8 * (st - 11))
                advance(0, 0, NP[0])
                advance(0, 1, NP[0])
                # start the next window before emitting proj so PE doesn't
                # stall on the previous window's epilogue chain
                advance(1, 0, 2)
                emit_proj(0)
                advance(1, 0, NP[1])
                advance(1, 1, NP[1])
                advance(2, 1, 2)
                emit_proj(1)
                advance(2, 1, NP[2])
                advance(2, 0, NP[2])
                emit_proj(2)

    return nc


def make_inputs(x, norm_w, norm_b, qkv_w, qkv_b, proj_w, proj_b):
    """Host-side prep: full-input numpy -> per-core in_maps."""
    x2 = np.ascontiguousarray(np.asarray(x, np.float32).reshape(C, SEQ))
    qkv_w = np.asarray(qkv_w, np.float32)
    qkv_b = np.asarray(qkv_b, np.float32)
    proj_w = np.asarray(proj_w, np.float32)
    proj_b = np.asarray(proj_b, np.float32)
    norm_w = np.asarray(norm_w, np.float32)
    norm_b = np.asarray(norm_b, np.float32)

    wT = np.ascontiguousarray(qkv_w.T)
    projT32 = np.ascontiguousarray(proj_w.T)
    # attn slice j holds heads (j, 2+j): permute projT rows to match
    perm = np.array([64 * (j + 2 * (p >= 64)) + (p % 64)
                     for j in range(2) for p in range(128)])
    projTb = np.ascontiguousarray(projT32[perm, :]).astype(ml_dtypes.bfloat16)
    qkvb = np.ascontiguousarray(qkv_b.reshape(6, 128).T)
    nwb = np.stack([norm_w[0:128], norm_w[128:256], norm_b[0:128], norm_b[128:256]], axis=1)
    projbrc = np.ascontiguousarray(proj_b.reshape(2, 128).T)
    cidx = np.arange(128)
    gidx = np.arange(16)
    G = ((cidx[:, None] // 8) == gidx[None, :]).astype(np.float32) / 8.0
    GT = np.ascontiguousarray(G.T * 8.0)

    xb = x2.astype(ml_dtypes.bfloat16)
    common = dict(x=xb, wT=wT, projTb=projTb, projT32=projT32, qkvb=qkvb,
                  nwb=nwb, projbrc=projbrc, G=G, GT=GT)
    in_maps = []
    cols = []
    for i in range(NCORES):
        ci = np.concatenate([
            np.arange(512 * i, 512 * (i + 1)),
            np.arange(4096 + 512 * i, 4096 + 512 * (i + 1)),
            np.arange(8192 + 128 * i, 8192 + 128 * (i + 1)),
        ])
        cols.append(ci)
        m = dict(common)
        m["xq"] = np.ascontiguousarray(x2[:, ci])
        in_maps.append(m)
    return in_maps, cols


_NC_CACHE = {}


def kernel(x, norm_w, norm_b, qkv_w, qkv_b, proj_w, proj_b):
    from concourse.bass_utils import run_bass_kernel_spmd

    _patch_tile_drain()
    _patch_to_json_split_waits()
    in_maps, cols = make_inputs(x, norm_w, norm_b, qkv_w, qkv_b, proj_w, proj_b)
    if "nc" not in _NC_CACHE:
        _NC_CACHE["nc"] = build_nc()
    nc = _NC_CACHE["nc"]
    res = run_bass_kernel_spmd(nc, in_maps, core_ids=list(range(NCORES)))
    out = np.zeros((C, SEQ), np.float32)
    for i in range(NCORES):
        out[:, cols[i]] = res.results[i]["out"]
    return out.reshape(1, C, 96, 96)



# revision 4
# speedup vs baseline: 1.2040x; 1.2040x over previous
"""Trainium2 Bass kernel for nn_AttentionBlock (GroupNorm + windowed MHA + proj + residual).

Contract: kernel(**inputs) takes FULL unsharded inputs (as from reference.setup_inputs())
and returns the FULL output [1, 256, 96, 96] float32.

Sharding: sequence-parallel over query positions across 8 cores. Each core gets a
uniform slice of each of the 3 reference attention windows:
  W0: q[512i   : 512(i+1)]    attends kv[0    : 6144]
  W1: q[4096+512i : ...]      attends kv[2048 : 9216]
  W2: q[8192+128i : ...]      attends kv[6144 : 9216]
All 4 heads computed locally; projection and residual local too. Every core
redundantly computes GroupNorm stats and full-sequence K/V.

v2 dataflow (single sweep, all 4 heads resident):
  - x [256, 9216] f32r loaded once into SBUF; bn_stats per tile as it arrives.
  - GroupNorm folded into the qkv weights: qkv(a*x+b) = (W*diag(a))x + W*b, so
    wT rows are scaled by a in place (one DVE op) and W*b lands in the per-dim
    bias vector; no xn tensor is ever materialized.
  - x shipped from host as bf16 (stats/kv-gen precision ~0.2%; residual and
    q-gen use the separate f32 xq input, so the residual is exact).
  - k/q stored fp8e4 in DoubleRow pair layout [32, 2, n] per head at
    partitions 32h (dim d -> partition d//2, slot d%2), produced by PSUM->fp8
    copy + 2-tile-batched SBUF fold-DMAs on the sync queue. v stored fp8e4
    [128 keys, chunk, head, 72], dims at 0:64 + ones column at 64.
  - QK and PV matmuls run fp8 DoubleRow (2 cols/cycle, contraction 64/256).
  - exp on ScalarE with scale=0.125, bias=-3.5 (shift cancels in softmax;
    keeps exp well inside fp8e4 range even with fp8 q/k noise - at -2 the
    hottest W0 logits saturated/overflowed), writing fp8 pair tiles that PV
    consumes directly. rstd = exp(-0.5*ln(var+eps)) keeps ACT on the one
    Ln/Exp table set (no ~2.7us table swaps).
  - sums via ones column of v; normalize by a rank-1 PE matmul broadcast of
    the reciprocal row (no DRAM bounce). attn slice j holds heads (j, 2+j)
    (projT rows host-permuted); heads 2,3 shift partitions via one DMA.
  - proj in bf16; v-bias (qkv_b_v + W_v b) folded into the output bias via
    pbtot = proj_b + P @ bias_v (runtime rank-1); residual from xq.

GroupNorm stats are estimated from every other seq tile (36864 samples per
group): estimator noise ~0.4% of sigma, raising end-to-end rel err from
2.3e-3 to 3.4e-3 (gate 2e-2) while halving the serial head-phase DVE.

Measured (repeat-slope metric, same harness for both): v1 baseline ~457us,
this version ~452 vs ~462us for full stats; cost-model sim: 393 -> 282us.
HW overlaps the PE->ACT->PE chain worse than the sim models (deletion
probes: exp marginal ~147us, PV ~40us, QK+qkv+head/tail ~248us). PSUM is
fully allocated (sps 2x2 + accps 2 + ops 2 = 8 banks), blocking deeper S
buffering and 4-chunk exp batches.
"""

import numpy as np
import ml_dtypes

import concourse.bass as bass
import concourse.tile as tile
from concourse import mybir
from concourse.vector_clock import ScopedClock, VectorClock

F32 = mybir.dt.float32
F32R = mybir.dt.float32r
BF16 = mybir.dt.bfloat16
FP8 = mybir.dt.float8e4
U8 = mybir.dt.uint8
AF = mybir.ActivationFunctionType
ALU = mybir.AluOpType
DR = mybir.MatmulPerfMode.DoubleRow

C = 256
SEQ = 9216
NCORES = 8
HEADS = 4
D = 64
EPS = 1e-5
SCALE = 0.125  # 1/sqrt(64)
EXPB = -3.5  # exp shift; cancels in softmax, keeps e^(s*SCALE+EXPB) well under 448 (fp8e4) even with fp8 q/k noise
# Schraudolph fp8e4m3 exp on DVE: byte = rint(s*SCH_A + SCH_B) saturating to
# [0,255]; bytes reinterpreted as fp8e4m3 give ~e^(s*SCALE+EXPB) with mantissa-
# linear error <6% - same order as fp8 quantization itself. HW f32->uint8
# convert is round-to-nearest + saturate (verified). Negative -> 0 (p=0 tail).
LOG2E = 1.4426950408889634
SCH_A = SCALE * LOG2E * 8.0
SCH_B = (EXPB * LOG2E + 7.0) * 8.0
# fraction of S->p conversions routed to DVE (Schraudolph) vs ACT (exact exp):
# pairs with pr % 8 in DVE_PRS go to DVE. ACT exp from PSUM runs ~990ns/pair
# vs DVE affine ~671ns; balance point keeps both engines ~equally busy.
DVE_PRS = frozenset((1, 3, 5))
NQC = 1152  # queries per core
ST = 512  # seq tile for qkv streaming
NST = SEQ // ST  # 18
NCH = SEQ // 128  # 72 key chunks
# windows: (q_off in NQC, q_len, key_chunk0, n_key_chunks)
WINDOWS = [(0, 512, 0, 48), (512, 512, 16, 56), (1024, 128, 48, 24)]


def input_shapes():
    """External-input name -> (shape, np dtype) for build_nc()'s signature."""
    return {
        "x": ((C, SEQ), ml_dtypes.bfloat16),
        "xq": ((C, NQC), np.float32),
        "wT": ((C, 3 * C), np.float32),
        "projTb": ((C, C), ml_dtypes.bfloat16),
        "projT32": ((C, C), np.float32),
        "qkvb": ((128, 6), np.float32),
        "nwb": ((128, 4), np.float32),
        "projbrc": ((128, 2), np.float32),
        "G": ((128, 16), np.float32),
        "GT": ((16, 128), np.float32),
    }


def _patch_tile_drain():
    """This container's walrus rejects >1 sem wait on one sync CTRL instruction
    ("Too many sync wait commands"). Split the TileContext-exit drain's waits
    into one-wait-per-nop instructions."""
    if getattr(tile.TileContext, "_drain_split_patched", False):
        return

    def _drain_and_barrier(self, tick_clock, wait_clock):
        vc = tick_clock.global_clock
        n = len(vc)
        for p in range(n):
            t = vc[p]
            if t <= 0:
                continue
            single = VectorClock([t if i == p else 0 for i in range(n)])
            inst = self.nc.sync.nop(nofuse=True, hint="drain_split_wait")
            wait_clock.add_sem_waits(inst.ins, ScopedClock({None: single}))
        self.nc.sync.drain()
        self.nc.all_engine_barrier()
        assert self.sems is not None
        popped = self.nc._tile_sem_poison_stack.pop()
        assert popped is self._sem_poison
        self.nc.clear_and_free_semaphores(list(self.sems.allocated().values()))
        self.nc.all_engine_barrier()

    tile.TileContext._drain_and_barrier = _drain_and_barrier
    tile.TileContext._drain_split_patched = True


def _patch_to_json_split_waits():
    """This walrus build rejects instructions carrying more than one sem-wait.
    Post-process the BIR JSON: keep one wait per instruction, move extras onto
    same-engine NoOps inserted just before it."""
    if getattr(bass.Bass, "_split_waits_patched", False):
        return
    import json as _json

    orig = bass.Bass.to_json_bytes

    def to_json_bytes(self):
        d = _json.loads(orig(self))
        for fn in d["functions"]:
            for blk in fn["blocks"]:
                out = []
                changed = False
                for ins in blk["instructions"]:
                    si = ins.get("sync_info")
                    waits = (si or {}).get("on_wait") or []
                    if len(waits) > 1:
                        changed = True
                        for k, w in enumerate(waits[:-1]):
                            out.append({
                                "debug": ins.get("debug", 0),
                                "engine": ins["engine"],
                                "ins": [],
                                "name": f"{ins['name']}-w{k}",
                                "opcode": "NoOp",
                                "outs": [],
                                "sync_info": {"on_update": [], "on_wait": [w]},
                                "text_hint": "split_wait",
                            })
                        si["on_wait"] = [waits[-1]]
                    out.append(ins)
                if changed:
                    blk["instructions"] = out
        return _json.dumps(d).encode()

    bass.Bass.to_json_bytes = to_json_bytes
    bass.Bass._split_waits_patched = True


def _bcast_free(ap, n):
    """[p, 1] AP -> [p, n, 1] AP broadcasting along a free dim (step 0)."""
    apl = [list(d) for d in ap.ap]
    return bass.AP(tensor=ap.tensor, offset=ap.offset, ap=[apl[0], [0, n]] + apl[1:])


def _bcast_part(ap, n):
    """[1, m] AP -> [n, m] AP broadcasting along partitions (step 0)."""
    apl = ap.ap
    assert apl[0][1] == 1, apl
    return bass.AP(tensor=ap.tensor, offset=ap.offset, ap=[[0, n]] + [list(d) for d in apl[1:]])


PROBE = None  # None | 'noexp' (skip exp+PV) | 'nopv' (skip PV)


def build_nc(repeat=1):
    nc = bass.Bass()

    x_d = nc.dram_tensor("x", [C, SEQ], BF16, kind="ExternalInput")
    xq_d = nc.dram_tensor("xq", [C, NQC], F32R, kind="ExternalInput")
    wT_d = nc.dram_tensor("wT", [C, 3 * C], F32R, kind="ExternalInput")
    projTb_d = nc.dram_tensor("projTb", [C, C], BF16, kind="ExternalInput")
    projT32_d = nc.dram_tensor("projT32", [C, C], F32R, kind="ExternalInput")
    qkvb_d = nc.dram_tensor("qkvb", [128, 6], F32, kind="ExternalInput")
    nwb_d = nc.dram_tensor("nwb", [128, 4], F32, kind="ExternalInput")
    projbrc_d = nc.dram_tensor("projbrc", [128, 2], F32, kind="ExternalInput")
    G_d = nc.dram_tensor("G", [128, 16], F32, kind="ExternalInput")
    GT_d = nc.dram_tensor("GT", [16, 128], F32, kind="ExternalInput")
    out_d = nc.dram_tensor("out", [C, NQC], F32, kind="ExternalOutput")

    with tile.TileContext(nc) as tc:
        for _rep in range(repeat):
            with (
                tc.tile_pool(name="singles", bufs=1) as singles,
                tc.tile_pool(name="stg", bufs=3) as stg,
                tc.tile_pool(name="pt", bufs=3) as ptp,
                tc.tile_pool(name="epi", bufs=3) as epi,
                tc.tile_pool(name="outp", bufs=2) as outp,
                tc.tile_pool(name="pg", bufs=4) as pg,
                tc.tile_pool(name="sps", bufs=2, space="PSUM") as sps,
                tc.tile_pool(name="accps", bufs=2, space="PSUM") as accps,
                tc.tile_pool(name="ops", bufs=2, space="PSUM") as ops,
                tc.tile_pool(name="dr", bufs=2, space="DRAM") as drp,
            ):
                # ---- x resident + streaming stats (loads first: keeps the
                # DMA queues clear of constant traffic) ----
                x_sb = singles.tile([128, 2, SEQ], BF16, tag="x_sb", name="x_sb")
                XC = 2048  # x dma chunk
                for b0 in range(0, SEQ, XC):
                    b1 = min(SEQ, b0 + XC)
                    nc.sync.dma_start(out=x_sb[:, 0, b0:b1], in_=x_d[0:128, b0:b1])
                    nc.gpsimd.dma_start(out=x_sb[:, 1, b0:b1], in_=x_d[128:256, b0:b1])

                # ---- constants ----
                wT_sb = singles.tile([128, 2, 3 * C], F32R, tag="wT")
                nc.sync.dma_start(out=wT_sb[:, 0, :], in_=wT_d[0:128, :])
                nc.sync.dma_start(out=wT_sb[:, 1, :], in_=wT_d[128:256, :])
                xq_sb = singles.tile([128, 2, NQC], F32R, tag="xq")
                nc.sync.dma_start(out=xq_sb[:, 0, :], in_=xq_d[0:128, :])
                nc.sync.dma_start(out=xq_sb[:, 1, :], in_=xq_d[128:256, :])
                G_sb = singles.tile([128, 16], F32, tag="G")
                nc.gpsimd.dma_start(out=G_sb, in_=G_d[:, :])
                GT_sb = singles.tile([16, 128], F32, tag="GT")
                nc.gpsimd.dma_start(out=GT_sb, in_=GT_d[:, :])
                qkvb_sb = singles.tile([128, 6], F32, tag="qkvb")
                nc.gpsimd.dma_start(out=qkvb_sb, in_=qkvb_d[:, :])
                nwb_sb = singles.tile([128, 4], F32, tag="nwb")
                nc.gpsimd.dma_start(out=nwb_sb, in_=nwb_d[:, :])
                projbrc_sb = singles.tile([128, 2], F32, tag="projbrc")
                nc.gpsimd.dma_start(out=projbrc_sb, in_=projbrc_d[:, :])
                projT32_sb = singles.tile([128, 2, C], F32R, tag="projT32")
                nc.gpsimd.dma_start(out=projT32_sb[:, 0, :], in_=projT32_d[0:128, :])
                nc.gpsimd.dma_start(out=projT32_sb[:, 1, :], in_=projT32_d[128:256, :])
                projTb_sb = singles.tile([128, 2, C], BF16, tag="projTb")
                nc.gpsimd.dma_start(out=projTb_sb[:, 0, :], in_=projTb_d[0:128, :])
                nc.gpsimd.dma_start(out=projTb_sb[:, 1, :], in_=projTb_d[128:256, :])

                # streaming stats (DVE, pipelined with the x chunks above).
                # Subsampled: every other tile (36864 samples per group) -
                # mean/rstd estimator noise ~0.4% of sigma, far inside the
                # accuracy budget, and it halves the serial head-phase DVE.
                NSS = NST // 2
                stats = singles.tile([128, 2, NSS, 6], F32, tag="stats")
                for si in range(NSS):
                    s0 = ST * 2 * si
                    for cc in range(2):
                        nc.vector.bn_stats(out=stats[:, cc, si, :], in_=x_sb[:, cc, s0:s0 + ST])

                epsb_sb = singles.tile([128, 1], F32, tag="epsb")
                nc.vector.memset(epsb_sb, EPS)
                warm = pg.tile([1, 1], F32, tag="warm")
                nc.vector.memset(warm, 0.0)
                nc.scalar.activation(out=warm, in_=warm, func=AF.Exp)

                # ---- GroupNorm a,b; fold into weights + biases ----
                ab_sb = singles.tile([128, 2, 2], F32, tag="ab")  # (a, b) per cc
                bvec_r = singles.tile([128, 2, 2], F32R, tag="bvec_r")  # b per cc, 2x dup (f32r)
                for cc in range(2):
                    mv = pg.tile([128, 2], F32, tag="mv")
                    nc.vector.bn_aggr(out=mv, in_=stats[:, cc, :, :])
                    st2 = pg.tile([128, 2], F32, tag="st2")  # (mean, E[x^2])
                    nc.vector.tensor_copy(out=st2[:, 0:1], in_=mv[:, 0:1])
                    nc.vector.tensor_tensor(out=st2[:, 1:2], in0=mv[:, 0:1], in1=mv[:, 0:1], op=ALU.mult)
                    nc.vector.tensor_tensor(out=st2[:, 1:2], in0=st2[:, 1:2], in1=mv[:, 1:2], op=ALU.add)
                    gps = accps.tile([128, 512], F32, tag="acc")
                    nc.tensor.matmul(gps[0:16, 0:2], lhsT=G_sb, rhs=st2, start=True, stop=True)
                    gm = pg.tile([16, 2], F32, tag="gm")  # (mean_g, E2_g)
                    nc.vector.tensor_copy(out=gm, in_=gps[0:16, 0:2])
                    t16 = pg.tile([16, 1], F32, tag="t16")
                    nc.vector.tensor_tensor(out=t16, in0=gm[:, 0:1], in1=gm[:, 0:1], op=ALU.mult)
                    nc.vector.tensor_tensor(out=gm[:, 1:2], in0=gm[:, 1:2], in1=t16, op=ALU.subtract)
                    # rstd = exp(-0.5*ln(var+eps)): stays in the Ln/Exp
                    # ACT table set, so no table swap before the big exps
                    nc.scalar.activation(out=gm[:, 1:2], in_=gm[:, 1:2], func=AF.Ln,
                                         bias=epsb_sb[0:16, :])
                    nc.scalar.activation(out=gm[:, 1:2], in_=gm[:, 1:2], func=AF.Exp,
                                         scale=-0.5)
                    mps = accps.tile([128, 512], F32, tag="acc")
                    nc.tensor.matmul(mps[0:128, 0:2], lhsT=GT_sb, rhs=gm, start=True, stop=True)
                    mr = pg.tile([128, 2], F32, tag="mr")  # (mean_c, rstd_c)
                    nc.vector.tensor_copy(out=mr, in_=mps[0:128, 0:2])
                    # a = rstd * norm_w ; b = norm_b - mean * a
                    nc.vector.tensor_tensor(out=ab_sb[:, cc, 0:1], in0=mr[:, 1:2], in1=nwb_sb[:, cc:cc + 1], op=ALU.mult)
                    t128 = pg.tile([128, 1], F32, tag="t128")
                    nc.vector.tensor_tensor(out=t128, in0=mr[:, 0:1], in1=ab_sb[:, cc, 0:1], op=ALU.mult)
                    nc.vector.tensor_tensor(out=ab_sb[:, cc, 1:2], in0=nwb_sb[:, 2 + cc:3 + cc], in1=t128, op=ALU.subtract)
                    nc.vector.tensor_copy(out=bvec_r[:, cc, :], in_=_bcast_free(ab_sb[:, cc, 1:2], 2))

                # bias_all = qkvb + W @ b  (uses UNSCALED wT; emitted before the
                # in-place scale below, so Tile orders the reads first)
                bps = accps.tile([128, 512], F32, tag="acc", name="bps")
                for s in range(6):
                    for cc in range(2):
                        nc.tensor.matmul(
                            bps[0:128, 2 * s:2 * s + 2], lhsT=wT_sb[:, cc, 128 * s:128 * (s + 1)],
                            rhs=bvec_r[:, cc, :],
                            start=(cc == 0), stop=(cc == 1))
                bias_all = singles.tile([128, 6], F32, tag="bias_all")
                nc.vector.tensor_tensor(out=bias_all, in0=qkvb_sb,
                                        in1=bps[0:128, 0:12].rearrange("p (s j) -> p s j", j=2)[:, :, 0],
                                        op=ALU.add)
                # pbtot = proj_b + P @ bias_v  (folds the v bias through proj)
                bvv = singles.tile([128, 2, 2], F32R, tag="bvv")
                for ds in range(2):
                    nc.vector.tensor_copy(out=bvv[:, ds, :], in_=_bcast_free(bias_all[:, 4 + ds:5 + ds], 2))
                pbps = accps.tile([128, 512], F32, tag="acc", name="pbps")
                for mc in range(2):
                    for ds in range(2):
                        nc.tensor.matmul(
                            pbps[0:128, 2 * mc:2 * mc + 2], lhsT=projT32_sb[:, ds, 128 * mc:128 * (mc + 1)],
                            rhs=bvv[:, ds, :], start=(ds == 0), stop=(ds == 1))
                pbtot = singles.tile([128, 2], F32, tag="pbtot")
                nc.vector.tensor_tensor(out=pbtot, in0=projbrc_sb,
                                        in1=pbps[0:128, 0:4].rearrange("p (s j) -> p s j", j=2)[:, :, 0],
                                        op=ALU.add)
                # scaled weight copies: bf16 (k/v-gen vs bf16 x) and f32r
                # q-slice (q-gen vs f32r xq)
                wTs_b = singles.tile([128, 2, 2 * C], BF16, tag="wTs_b")
                wTs_q = singles.tile([128, 2, C], F32R, tag="wTs_q")
                for cc in range(2):
                    nc.vector.tensor_scalar_mul(
                        out=wTs_b[:, cc, :], in0=wT_sb[:, cc, C:3 * C],
                        scalar1=ab_sb[:, cc, 0:1])
                    nc.vector.tensor_scalar_mul(
                        out=wTs_q[:, cc, :], in0=wT_sb[:, cc, 0:C],
                        scalar1=ab_sb[:, cc, 0:1])

                # ---- fp8 k/v/q caches ----
                # k: [p, slot, key] per head at partitions 32h..32h+32; dim d of
                # head -> partition 32h + d//2, slot d%2
                k_sb = singles.tile([128, 2, SEQ], FP8, tag="k_sb", name="k_sb")
                # v: [key-in-chunk, chunk, head, 72]; dims at 0:64, ones at 64
                v_sb = singles.tile([128, NCH, HEADS, 72], FP8, tag="v_sb", name="v_sb")
                ones_f = singles.tile([128, 1], F32, tag="ones_f")
                nc.vector.memset(ones_f, 1.0)
                ones_v_bc = bass.AP(tensor=ones_f.tensor, offset=ones_f.offset,
                                    ap=[list(ones_f.ap[0]), [0, NCH], [0, HEADS], [1, 1]])
                nc.vector.tensor_copy(out=v_sb[:, :, :, 64:65], in_=ones_v_bc)
                q_sb = singles.tile([128, 2, NQC], FP8, tag="q_sb", name="q_sb")
                expb_sb = singles.tile([128, 1], F32, tag="expb")
                nc.vector.memset(expb_sb, EXPB)
                ones_r = singles.tile([1, 64], F32R, tag="ones_r")
                nc.vector.tensor_copy(out=ones_r, in_=_bcast_free(ones_f[0:1, 0:1], 64))

                # ---- q generation (per window, both dim-slices) ----
                def emit_q(wsel):
                    for s in range(2):  # dim slice: heads (0,1) / (2,3)
                        for w, (q0, qn, _, _) in ((w, WINDOWS[w]) for w in wsel):
                            qps = accps.tile([128, 512], F32, tag="acc", name="qps")
                            for cc in range(2):
                                nc.tensor.matmul(
                                    qps[:, 0:qn],
                                    lhsT=wTs_q[:, cc, 128 * s:128 * s + 128],
                                    rhs=xq_sb[:, cc, q0:q0 + qn], start=(cc == 0), stop=(cc == 1))
                            qstage = stg.tile([128, 512], FP8, tag="stage", name="qstage")
                            nc.vector.tensor_scalar_add(
                                out=qstage[:, 0:qn], in0=qps[:, 0:qn], scalar1=bias_all[:, s:s + 1])
                            for hp in range(2):  # head within slice
                                h = 2 * s + hp
                                eng = nc.sync if hp == 0 else nc.gpsimd
                                eng.dma_start(
                                    out=q_sb[32 * h:32 * h + 32, :, q0:q0 + qn],
                                    in_=qstage[64 * hp:64 * hp + 64, 0:qn])

                # ---- k/v generation for one seq tile ----
                # k staged fp8 per slice in 2-tile groups; one fold-DMA per
                # head per group ([64, 1024] contiguous -> cheap descriptors)
                kstage2 = {}

                def emit_kv_tile(st):
                    s0 = ST * st
                    g = st % 2
                    for s in range(2):  # k dim slice: heads (0,1) / (2,3)
                        kps = accps.tile([128, 512], F32, tag="acc", name="kps")
                        for cc in range(2):
                            nc.tensor.matmul(
                                kps, lhsT=wTs_b[:, cc, 128 * s:128 * s + 128],
                                rhs=x_sb[:, cc, s0:s0 + ST], start=(cc == 0), stop=(cc == 1))
                        if g == 0:
                            kstage2[s] = stg.tile([128, 2, 512], FP8, tag=f"stage{s}", name="kstage2")
                        nc.vector.tensor_scalar_add(
                            out=kstage2[s][:, g, :], in0=kps, scalar1=bias_all[:, 2 + s:3 + s])
                        if g == 1:
                            f0 = ST * (st - 1)
                            for hp in range(2):
                                h = 2 * s + hp
                                nc.sync.dma_start(
                                    out=k_sb[32 * h:32 * h + 32, :, f0:f0 + 2 * ST],
                                    in_=kstage2[s][64 * hp:64 * hp + 64, :, :])
                    # v: 4 token chunks of 128; out [128 tokens, 256 dims]
                    vps = accps.tile([128, 512], F32, tag="acc", name="vps")
                    for mc in range(4):
                        o = 256 * (mc % 2)
                        for cc in range(2):
                            nc.tensor.matmul(
                                vps[:, o:o + 256],
                                lhsT=x_sb[:, cc, s0 + 128 * mc:s0 + 128 * (mc + 1)],
                                rhs=wTs_b[:, cc, C:2 * C], start=(cc == 0), stop=(cc == 1))
                        ch = 4 * st + mc
                        vv = vps[:, o:o + 256].rearrange("p (h d) -> p h d", h=4)
                        nc.vector.tensor_copy(out=v_sb[:, ch, :, 0:64], in_=vv)

                # ---- attention stream for one (window, head) ----
                # attn layout: slice j holds heads (j, 2+j); head h -> slice h%2,
                # partitions 64*(h//2). projTb rows are host-permuted to match.
                def gen_stream(w, h):
                    q0, qn, kc0, nch = WINDOWS[w]
                    o_t = ops.tile([128, 512], F32, tag="o", name="o_t") if not PROBE else None
                    npair = nch // 2
                    for pr in range(npair):
                        c0 = kc0 + 2 * pr
                        s_ps = sps.tile([128, 2, 512], F32, tag="s", name="s_ps")
                        for j in range(2):
                            nc.tensor.matmul(
                                s_ps[:, j, 0:qn],
                                lhsT=k_sb[32 * h:32 * h + 32, :, 128 * (c0 + j):128 * (c0 + j) + 128],
                                rhs=q_sb[32 * h:32 * h + 32, :, q0:q0 + qn],
                                start=True, stop=True, perf_mode=DR,
                                tile_position=(32 * h, 0))
                        if PROBE != 'noexp':
                            pt = ptp.tile([128, 2, 512], FP8, tag="p", name="pt")
                            if pr % 8 in DVE_PRS:
                                nc.vector.tensor_scalar(
                                    out=pt[:, :, 0:qn].bitcast(U8), in0=s_ps[:, :, 0:qn],
                                    scalar1=SCH_A, scalar2=SCH_B,
                                    op0=ALU.mult, op1=ALU.add)
                            else:
                                nc.scalar.activation(out=pt[:, :, 0:qn], in_=s_ps[:, :, 0:qn],
                                                     func=AF.Exp, scale=SCALE, bias=expb_sb)
                            if not PROBE:
                                nc.tensor.matmul(
                                    o_t[0:65, 0:qn],
                                    lhsT=v_sb[:, c0:c0 + 2, h, 0:65],
                                    rhs=pt[:, :, 0:qn],
                                    start=(pr == 0), stop=(pr == npair - 1), perf_mode=DR)
                        yield
                    if PROBE:
                        dbase = 64 * (h // 2)
                        nc.vector.tensor_copy(
                            out=attn_sb[dbase:dbase + 64, h % 2, q0:q0 + qn],
                            in_=_bcast_free(ones_f[0:64, 0:1], qn))
                        return
                    # epilogue: normalize by reciprocal of sums row 64,
                    # partition-broadcast via a rank-1 PE matmul (ones x rec)
                    osb = epi.tile([65, 512], F32, tag="osb", name="osb")
                    nc.vector.tensor_copy(out=osb[:, 0:qn], in_=o_t[0:65, 0:qn])
                    rec = epi.tile([1, 512], F32R, tag="rec", name="rec")
                    with nc.allow_low_precision(reason="f32r out is bit-identical to f32"):
                        nc.vector.reciprocal(out=rec[0:1, 0:qn], in_=osb[64:65, 0:qn])
                    recb = accps.tile([128, 512], F32, tag="acc", name="recb")
                    nc.tensor.matmul(recb[0:64, 0:qn], lhsT=ones_r, rhs=rec[0:1, 0:qn],
                                     start=True, stop=True)
                    if h < 2:
                        nc.vector.tensor_tensor(
                            out=attn_sb[0:64, h, q0:q0 + qn],
                            in0=osb[0:64, 0:qn], in1=recb[0:64, 0:qn], op=ALU.mult)
                    else:
                        at = epi.tile([64, 512], BF16, tag="at", name="at")
                        nc.vector.tensor_tensor(
                            out=at[:, 0:qn], in0=osb[0:64, 0:qn], in1=recb[0:64, 0:qn], op=ALU.mult)
                        nc.sync.dma_start(out=attn_sb[64:128, h - 2, q0:q0 + qn], in_=at[:, 0:qn])

                attn_sb = singles.tile([128, 2, NQC], BF16, tag="attn", name="attn_sb")

                def emit_proj(w):
                    q0, qn, _, _ = WINDOWS[w]
                    for mc in range(2):
                        pp = accps.tile([128, 512], F32, tag="acc", name="pp")
                        for ds in range(2):
                            nc.tensor.matmul(
                                pp[:, 0:qn], lhsT=projTb_sb[:, ds, 128 * mc:128 * (mc + 1)],
                                rhs=attn_sb[:, ds, q0:q0 + qn], start=(ds == 0), stop=(ds == 1))
                        ot = outp.tile([128, 512], F32, tag="ot", name="ot")
                        nc.vector.tensor_scalar_add(out=ot[:, 0:qn], in0=pp[:, 0:qn],
                                                    scalar1=pbtot[:, mc:mc + 1])
                        nc.vector.tensor_tensor(out=ot[:, 0:qn], in0=ot[:, 0:qn],
                                                in1=xq_sb[:, mc, q0:q0 + qn], op=ALU.add)
                        nc.sync.dma_start(out=out_d[128 * mc:128 * (mc + 1), q0:q0 + qn], in_=ot[:, 0:qn])

                # ---- schedule ----
                emit_q([0])

                # head-pair round-robin streams per window, gated on kv tiles.
                # stream (w,h) pair pr needs chunks kc0+2pr+1 <= last emitted
                def ready_pairs(w, last_chunk):
                    _, _, kc0, nch = WINDOWS[w]
                    return min(nch // 2, max(0, (last_chunk - kc0 - 1) // 2 + 1))

                # phase 1: emit kv tiles, interleaving W0 streams for heads 0,1
                # then 2,3; then W1, W2 the same way once their chunks exist.
                gens = {}
                done_pairs = {}
                for w in range(3):
                    for h in range(HEADS):
                        gens[(w, h)] = gen_stream(w, h)
                        done_pairs[(w, h)] = 0

                def advance(w, hpair, upto):
                    """Advance streams (w, 2*hpair) and (w, 2*hpair+1) round-robin
                    until each has emitted `upto` pairs (or exhausted)."""
                    hs = (2 * hpair, 2 * hpair + 1)
                    while True:
                        progressed = False
                        for h in hs:
                            if done_pairs[(w, h)] < upto:
                                try:
                                    next(gens[(w, h)])
                                    done_pairs[(w, h)] += 1
                                    progressed = True
                                except StopIteration:
                                    done_pairs[(w, h)] = 10 ** 9
                        if not progressed:
                            break

                # +1: the final next() runs the epilogue (StopIteration)
                NP = [WINDOWS[w][3] // 2 + 1 for w in range(3)]
                for st in range(NST):
                    emit_kv_tile(st)
                    if st == 2:
                        emit_q([1])
                    elif st == 4:
                        emit_q([2])
                    if st % 2 == 0:
                        continue  # chunks land with the fold at odd st
                    last_chunk = 4 * st + 3
                    # W0 heads 0,1 track the kv frontier; heads 2,3 fill the
                    # tail tiles (W0 chunks all ready from tile 11)
                    advance(0, 0, ready_pairs(0, last_chunk))
                    if st >= 11:
                        advance(0, 0, NP[0])  # epilogues before heads 2,3 start
                    if st >= 13:
                        advance(0, 1, 8 * (st - 11))
                advance(0, 0, NP[0])
                advance(0, 1, NP[0])
                # start the next window before emitting proj so PE doesn't
                # stall on the previous window's epilogue chain
                advance(1, 0, 2)
                emit_proj(0)
                advance(1, 0, NP[1])
                advance(1, 1, NP[1])
                advance(2, 1, 2)
                emit_proj(1)
                advance(2, 1, NP[2])
                advance(2, 0, NP[2])
                emit_proj(2)

    return nc


def make_inputs(x, norm_w, norm_b, qkv_w, qkv_b, proj_w, proj_b):
    """Host-side prep: full-input numpy -> per-core in_maps."""
    x2 = np.ascontiguousarray(np.asarray(x, np.float32).reshape(C, SEQ))
    qkv_w = np.asarray(qkv_w, np.float32)
    qkv_b = np.asarray(qkv_b, np.float32)
    proj_w = np.asarray(proj_w, np.float32)
    proj_b = np.asarray(proj_b, np.float32)
    norm_w = np.asarray(norm_w, np.float32)
    norm_b = np.asarray(norm_b, np.float32)

    wT = np.ascontiguousarray(qkv_w.T)
    projT32 = np.ascontiguousarray(proj_w.T)
    # attn slice j holds heads (j, 2+j): permute projT rows to match
    perm = np.array([64 * (j + 2 * (p >= 64)) + (p % 64)
                     for j in range(2) for p in range(128)])
    projTb = np.ascontiguousarray(projT32[perm, :]).astype(ml_dtypes.bfloat16)
    qkvb = np.ascontiguousarray(qkv_b.reshape(6, 128).T)
    nwb = np.stack([norm_w[0:128], norm_w[128:256], norm_b[0:128], norm_b[128:256]], axis=1)
    projbrc = np.ascontiguousarray(proj_b.reshape(2, 128).T)
    cidx = np.arange(128)
    gidx = np.arange(16)
    G = ((cidx[:, None] // 8) == gidx[None, :]).astype(np.float32) / 8.0
    GT = np.ascontiguousarray(G.T * 8.0)

    xb = x2.astype(ml_dtypes.bfloat16)
    common = dict(x=xb, wT=wT, projTb=projTb, projT32=projT32, qkvb=qkvb,
                  nwb=nwb, projbrc=projbrc, G=G, GT=GT)
    in_maps = []
    cols = []
    for i in range(NCORES):
        ci = np.concatenate([
            np.arange(512 * i, 512 * (i + 1)),
            np.arange(4096 + 512 * i, 4096 + 512 * (i + 1)),
            np.arange(8192 + 128 * i, 8192 + 128 * (i + 1)),
        ])
        cols.append(ci)
        m = dict(common)
        m["xq"] = np.ascontiguousarray(x2[:, ci])
        in_maps.append(m)
    return in_maps, cols


_NC_CACHE = {}


def kernel(x, norm_w, norm_b, qkv_w, qkv_b, proj_w, proj_b):
    from concourse.bass_utils import run_bass_kernel_spmd

    _patch_tile_drain()
    _patch_to_json_split_waits()
    in_maps, cols = make_inputs(x, norm_w, norm_b, qkv_w, qkv_b, proj_w, proj_b)
    if "nc" not in _NC_CACHE:
        _NC_CACHE["nc"] = build_nc()
    nc = _NC_CACHE["nc"]
    res = run_bass_kernel_spmd(nc, in_maps, core_ids=list(range(NCORES)))
    out = np.zeros((C, SEQ), np.float32)
    for i in range(NCORES):
        out[:, cols[i]] = res.results[i]["out"]
    return out.reshape(1, C, 96, 96)



# revision 8
# speedup vs baseline: 1.2704x; 1.0551x over previous
"""Trainium2 Bass kernel for nn_AttentionBlock (GroupNorm + windowed MHA + proj + residual).

Contract: kernel(**inputs) takes FULL unsharded inputs (as from reference.setup_inputs())
and returns the FULL output [1, 256, 96, 96] float32.

Sharding: sequence-parallel over query positions across 8 cores. Each core gets a
uniform slice of each of the 3 reference attention windows:
  W0: q[512i   : 512(i+1)]    attends kv[0    : 6144]
  W1: q[4096+512i : ...]      attends kv[2048 : 9216]
  W2: q[8192+128i : ...]      attends kv[6144 : 9216]
All 4 heads computed locally; projection and residual local too. Every core
redundantly computes GroupNorm stats and full-sequence K/V.

v2 dataflow (single sweep, all 4 heads resident):
  - x [256, 9216] f32r loaded once into SBUF; bn_stats per tile as it arrives.
  - GroupNorm folded into the qkv weights: qkv(a*x+b) = (W*diag(a))x + W*b, so
    wT rows are scaled by a in place (one DVE op) and W*b lands in the per-dim
    bias vector; no xn tensor is ever materialized.
  - x shipped from host as bf16 (stats/kv-gen precision ~0.2%; residual and
    q-gen use the separate f32 xq input, so the residual is exact).
  - k/q stored fp8e4 in DoubleRow pair layout [32, 2, n] per head at
    partitions 32h (dim d -> partition d//2, slot d%2), produced by PSUM->fp8
    copy + 2-tile-batched SBUF fold-DMAs on the sync queue. v stored fp8e4
    [128 keys, chunk, head, 72], dims at 0:64 + ones column at 64.
  - QK and PV matmuls run fp8 DoubleRow (2 cols/cycle, contraction 64/256).
  - exp on ScalarE with scale=0.125, bias=-3.5 (shift cancels in softmax;
    keeps exp well inside fp8e4 range even with fp8 q/k noise - at -2 the
    hottest W0 logits saturated/overflowed), writing fp8 pair tiles that PV
    consumes directly. rstd = exp(-0.5*ln(var+eps)) keeps ACT on the one
    Ln/Exp table set (no ~2.7us table swaps).
  - sums via ones column of v; normalize by a rank-1 PE matmul broadcast of
    the reciprocal row (no DRAM bounce). attn slice j holds heads (j, 2+j)
    (projT rows host-permuted); heads 2,3 shift partitions via one DMA.
  - proj in bf16; v-bias (qkv_b_v + W_v b) folded into the output bias via
    pbtot = proj_b + P @ bias_v (runtime rank-1); residual from xq.

GroupNorm stats are estimated from every other seq tile (36864 samples per
group): estimator noise ~0.4% of sigma, raising end-to-end rel err from
2.3e-3 to 3.4e-3 (gate 2e-2) while halving the serial head-phase DVE.

Measured (repeat-slope metric, same harness for both): v1 baseline ~457us,
this version ~452 vs ~462us for full stats; cost-model sim: 393 -> 282us.
HW overlaps the PE->ACT->PE chain worse than the sim models (deletion
probes: exp marginal ~147us, PV ~40us, QK+qkv+head/tail ~248us). PSUM is
fully allocated (sps 2x2 + accps 2 + ops 2 = 8 banks), blocking deeper S
buffering and 4-chunk exp batches.
"""

import numpy as np
import ml_dtypes

import concourse.bass as bass
import concourse.tile as tile
from concourse import mybir
from concourse.vector_clock import ScopedClock, VectorClock

F32 = mybir.dt.float32
F32R = mybir.dt.float32r
BF16 = mybir.dt.bfloat16
FP8 = mybir.dt.float8e4
U8 = mybir.dt.uint8
AF = mybir.ActivationFunctionType
ALU = mybir.AluOpType
DR = mybir.MatmulPerfMode.DoubleRow

C = 256
SEQ = 9216
NCORES = 8
HEADS = 4
D = 64
EPS = 1e-5
SCALE = 0.125  # 1/sqrt(64)
EXPB = -3.5  # exp shift; cancels in softmax, keeps e^(s*SCALE+EXPB) well under 448 (fp8e4) even with fp8 q/k noise
# Schraudolph fp8e4m3 exp on DVE: byte = rint(s*SCH_A + SCH_B) saturating to
# [0,255]; bytes reinterpreted as fp8e4m3 give ~e^(s*SCALE+EXPB) with mantissa-
# linear error <6% - same order as fp8 quantization itself. HW f32->uint8
# convert is round-to-nearest + saturate (verified). Negative -> 0 (p=0 tail).
LOG2E = 1.4426950408889634
SCH_A = SCALE * LOG2E * 8.0
SCH_B = (EXPB * LOG2E + 7.0) * 8.0
# fraction of S->p conversions routed to DVE (Schraudolph) vs ACT (exact exp):
# pairs with pr % 8 in DVE_PRS go to DVE. ACT exp from PSUM runs ~990ns/pair
# vs DVE affine ~671ns; balance point keeps both engines ~equally busy.
DVE_PRS = frozenset((1, 3, 5))
NQC = 1152  # queries per core
ST = 512  # seq tile for qkv streaming
NST = SEQ // ST  # 18
NCH = SEQ // 128  # 72 key chunks
# windows: (q_off in NQC, q_len, key_chunk0, n_key_chunks)
WINDOWS = [(0, 512, 0, 48), (512, 512, 16, 56), (1024, 128, 48, 24)]


def input_shapes():
    """External-input name -> (shape, np dtype) for build_nc()'s signature."""
    return {
        "x": ((C, SEQ), ml_dtypes.bfloat16),
        "xq": ((C, NQC), np.float32),
        "wT": ((C, 3 * C), np.float32),
        "projTb": ((C, C), ml_dtypes.bfloat16),
        "projT32": ((C, C), np.float32),
        "qkvb": ((128, 6), np.float32),
        "nwb": ((128, 4), np.float32),
        "projbrc": ((128, 2), np.float32),
        "G": ((128, 16), np.float32),
        "GT": ((16, 128), np.float32),
    }


def _patch_tile_drain():
    """This container's walrus rejects >1 sem wait on one sync CTRL instruction
    ("Too many sync wait commands"). Split the TileContext-exit drain's waits
    into one-wait-per-nop instructions."""
    if getattr(tile.TileContext, "_drain_split_patched", False):
        return

    def _drain_and_barrier(self, tick_clock, wait_clock):
        vc = tick_clock.global_clock
        n = len(vc)
        for p in range(n):
            t = vc[p]
            if t <= 0:
                continue
            single = VectorClock([t if i == p else 0 for i in range(n)])
            inst = self.nc.sync.nop(nofuse=True, hint="drain_split_wait")
            wait_clock.add_sem_waits(inst.ins, ScopedClock({None: single}))
        self.nc.sync.drain()
        self.nc.all_engine_barrier()
        assert self.sems is not None
        popped = self.nc._tile_sem_poison_stack.pop()
        assert popped is self._sem_poison
        self.nc.clear_and_free_semaphores(list(self.sems.allocated().values()))
        self.nc.all_engine_barrier()

    tile.TileContext._drain_and_barrier = _drain_and_barrier
    tile.TileContext._drain_split_patched = True


def _patch_to_json_split_waits():
    """This walrus build rejects instructions carrying more than one sem-wait.
    Post-process the BIR JSON: keep one wait per instruction, move extras onto
    same-engine NoOps inserted just before it."""
    if getattr(bass.Bass, "_split_waits_patched", False):
        return
    import json as _json

    orig = bass.Bass.to_json_bytes

    def to_json_bytes(self):
        d = _json.loads(orig(self))
        for fn in d["functions"]:
            for blk in fn["blocks"]:
                out = []
                changed = False
                for ins in blk["instructions"]:
                    si = ins.get("sync_info")
                    waits = (si or {}).get("on_wait") or []
                    if len(waits) > 1:
                        changed = True
                        for k, w in enumerate(waits[:-1]):
                            out.append({
                                "debug": ins.get("debug", 0),
                                "engine": ins["engine"],
                                "ins": [],
                                "name": f"{ins['name']}-w{k}",
                                "opcode": "NoOp",
                                "outs": [],
                                "sync_info": {"on_update": [], "on_wait": [w]},
                                "text_hint": "split_wait",
                            })
                        si["on_wait"] = [waits[-1]]
                    out.append(ins)
                if changed:
                    blk["instructions"] = out
        return _json.dumps(d).encode()

    bass.Bass.to_json_bytes = to_json_bytes
    bass.Bass._split_waits_patched = True


def _bcast_free(ap, n):
    """[p, 1] AP -> [p, n, 1] AP broadcasting along a free dim (step 0)."""
    apl = [list(d) for d in ap.ap]
    return bass.AP(tensor=ap.tensor, offset=ap.offset, ap=[apl[0], [0, n]] + apl[1:])


def _bcast_part(ap, n):
    """[1, m] AP -> [n, m] AP broadcasting along partitions (step 0)."""
    apl = ap.ap
    assert apl[0][1] == 1, apl
    return bass.AP(tensor=ap.tensor, offset=ap.offset, ap=[[0, n]] + [list(d) for d in apl[1:]])


PROBE = None  # None | 'noexp' (skip exp+PV) | 'nopv' (skip PV)


def build_nc(repeat=1):
    nc = bass.Bass()

    x_d = nc.dram_tensor("x", [C, SEQ], BF16, kind="ExternalInput")
    xq_d = nc.dram_tensor("xq", [C, NQC], F32R, kind="ExternalInput")
    wT_d = nc.dram_tensor("wT", [C, 3 * C], F32R, kind="ExternalInput")
    projTb_d = nc.dram_tensor("projTb", [C, C], BF16, kind="ExternalInput")
    projT32_d = nc.dram_tensor("projT32", [C, C], F32R, kind="ExternalInput")
    qkvb_d = nc.dram_tensor("qkvb", [128, 6], F32, kind="ExternalInput")
    nwb_d = nc.dram_tensor("nwb", [128, 4], F32, kind="ExternalInput")
    projbrc_d = nc.dram_tensor("projbrc", [128, 2], F32, kind="ExternalInput")
    G_d = nc.dram_tensor("G", [128, 16], F32, kind="ExternalInput")
    GT_d = nc.dram_tensor("GT", [16, 128], F32, kind="ExternalInput")
    out_d = nc.dram_tensor("out", [C, NQC], F32, kind="ExternalOutput")

    with tile.TileContext(nc) as tc:
        for _rep in range(repeat):
            with (
                tc.tile_pool(name="singles", bufs=1) as singles,
                tc.tile_pool(name="stg", bufs=3) as stg,
                tc.tile_pool(name="pt", bufs=3) as ptp,
                tc.tile_pool(name="epi", bufs=3) as epi,
                tc.tile_pool(name="outp", bufs=2) as outp,
                tc.tile_pool(name="pg", bufs=4) as pg,
                tc.tile_pool(name="sps", bufs=2, space="PSUM") as sps,
                tc.tile_pool(name="accps", bufs=2, space="PSUM") as accps,
                tc.tile_pool(name="ops", bufs=2, space="PSUM") as ops,
                tc.tile_pool(name="dr", bufs=2, space="DRAM") as drp,
            ):
                # ---- x resident + streaming stats (loads first: keeps the
                # DMA queues clear of constant traffic) ----
                x_sb = singles.tile([128, 2, SEQ], BF16, tag="x_sb", name="x_sb")
                XC = 2048  # x dma chunk
                for b0 in range(0, SEQ, XC):
                    b1 = min(SEQ, b0 + XC)
                    nc.sync.dma_start(out=x_sb[:, 0, b0:b1], in_=x_d[0:128, b0:b1])
                    nc.gpsimd.dma_start(out=x_sb[:, 1, b0:b1], in_=x_d[128:256, b0:b1])

                # ---- constants ----
                wT_sb = singles.tile([128, 2, 3 * C], F32R, tag="wT")
                nc.sync.dma_start(out=wT_sb[:, 0, :], in_=wT_d[0:128, :])
                nc.sync.dma_start(out=wT_sb[:, 1, :], in_=wT_d[128:256, :])
                xq_sb = singles.tile([128, 2, NQC], F32R, tag="xq")
                nc.sync.dma_start(out=xq_sb[:, 0, :], in_=xq_d[0:128, :])
                nc.sync.dma_start(out=xq_sb[:, 1, :], in_=xq_d[128:256, :])
                G_sb = singles.tile([128, 16], F32, tag="G")
                nc.gpsimd.dma_start(out=G_sb, in_=G_d[:, :])
                GT_sb = singles.tile([16, 128], F32, tag="GT")
                nc.gpsimd.dma_start(out=GT_sb, in_=GT_d[:, :])
                qkvb_sb = singles.tile([128, 6], F32, tag="qkvb")
                nc.gpsimd.dma_start(out=qkvb_sb, in_=qkvb_d[:, :])
                nwb_sb = singles.tile([128, 4], F32, tag="nwb")
                nc.gpsimd.dma_start(out=nwb_sb, in_=nwb_d[:, :])
                projbrc_sb = singles.tile([128, 2], F32, tag="projbrc")
                nc.gpsimd.dma_start(out=projbrc_sb, in_=projbrc_d[:, :])
                projT32_sb = singles.tile([128, 2, C], F32R, tag="projT32")
                nc.gpsimd.dma_start(out=projT32_sb[:, 0, :], in_=projT32_d[0:128, :])
                nc.gpsimd.dma_start(out=projT32_sb[:, 1, :], in_=projT32_d[128:256, :])
                projTb_sb = singles.tile([128, 2, C], BF16, tag="projTb")
                nc.gpsimd.dma_start(out=projTb_sb[:, 0, :], in_=projTb_d[0:128, :])
                nc.gpsimd.dma_start(out=projTb_sb[:, 1, :], in_=projTb_d[128:256, :])

                # streaming stats (DVE, pipelined with the x chunks above).
                # Subsampled: every other tile (36864 samples per group) -
                # mean/rstd estimator noise ~0.4% of sigma, far inside the
                # accuracy budget, and it halves the serial head-phase DVE.
                NSS = NST // 2
                stats = singles.tile([128, 2, NSS, 6], F32, tag="stats")
                for si in range(NSS):
                    s0 = ST * 2 * si
                    for cc in range(2):
                        nc.vector.bn_stats(out=stats[:, cc, si, :], in_=x_sb[:, cc, s0:s0 + ST])

                epsb_sb = singles.tile([128, 1], F32, tag="epsb")
                nc.vector.memset(epsb_sb, EPS)
                warm = pg.tile([1, 1], F32, tag="warm")
                nc.vector.memset(warm, 0.0)
                nc.scalar.activation(out=warm, in_=warm, func=AF.Exp)

                # ---- GroupNorm a,b; fold into weights + biases ----
                ab_sb = singles.tile([128, 2, 2], F32, tag="ab")  # (a, b) per cc
                bvec_r = singles.tile([128, 2, 2], F32R, tag="bvec_r")  # b per cc, 2x dup (f32r)
                for cc in range(2):
                    mv = pg.tile([128, 2], F32, tag="mv")
                    nc.vector.bn_aggr(out=mv, in_=stats[:, cc, :, :])
                    st2 = pg.tile([128, 2], F32, tag="st2")  # (mean, E[x^2])
                    nc.vector.tensor_copy(out=st2[:, 0:1], in_=mv[:, 0:1])
                    nc.vector.tensor_tensor(out=st2[:, 1:2], in0=mv[:, 0:1], in1=mv[:, 0:1], op=ALU.mult)
                    nc.vector.tensor_tensor(out=st2[:, 1:2], in0=st2[:, 1:2], in1=mv[:, 1:2], op=ALU.add)
                    gps = accps.tile([128, 512], F32, tag="acc")
                    nc.tensor.matmul(gps[0:16, 0:2], lhsT=G_sb, rhs=st2, start=True, stop=True)
                    gm = pg.tile([16, 2], F32, tag="gm")  # (mean_g, E2_g)
                    nc.vector.tensor_copy(out=gm, in_=gps[0:16, 0:2])
                    t16 = pg.tile([16, 1], F32, tag="t16")
                    nc.vector.tensor_tensor(out=t16, in0=gm[:, 0:1], in1=gm[:, 0:1], op=ALU.mult)
                    nc.vector.tensor_tensor(out=gm[:, 1:2], in0=gm[:, 1:2], in1=t16, op=ALU.subtract)
                    # rstd = exp(-0.5*ln(var+eps)): stays in the Ln/Exp
                    # ACT table set, so no table swap before the big exps
                    nc.scalar.activation(out=gm[:, 1:2], in_=gm[:, 1:2], func=AF.Ln,
                                         bias=epsb_sb[0:16, :])
                    nc.scalar.activation(out=gm[:, 1:2], in_=gm[:, 1:2], func=AF.Exp,
                                         scale=-0.5)
                    mps = accps.tile([128, 512], F32, tag="acc")
                    nc.tensor.matmul(mps[0:128, 0:2], lhsT=GT_sb, rhs=gm, start=True, stop=True)
                    mr = pg.tile([128, 2], F32, tag="mr")  # (mean_c, rstd_c)
                    nc.vector.tensor_copy(out=mr, in_=mps[0:128, 0:2])
                    # a = rstd * norm_w ; b = norm_b - mean * a
                    nc.vector.tensor_tensor(out=ab_sb[:, cc, 0:1], in0=mr[:, 1:2], in1=nwb_sb[:, cc:cc + 1], op=ALU.mult)
                    t128 = pg.tile([128, 1], F32, tag="t128")
                    nc.vector.tensor_tensor(out=t128, in0=mr[:, 0:1], in1=ab_sb[:, cc, 0:1], op=ALU.mult)
                    nc.vector.tensor_tensor(out=ab_sb[:, cc, 1:2], in0=nwb_sb[:, 2 + cc:3 + cc], in1=t128, op=ALU.subtract)
                    nc.vector.tensor_copy(out=bvec_r[:, cc, :], in_=_bcast_free(ab_sb[:, cc, 1:2], 2))

                # bias_all = qkvb + W @ b  (uses UNSCALED wT; emitted before the
                # in-place scale below, so Tile orders the reads first)
                bps = accps.tile([128, 512], F32, tag="acc", name="bps")
                for s in range(6):
                    for cc in range(2):
                        nc.tensor.matmul(
                            bps[0:128, 2 * s:2 * s + 2], lhsT=wT_sb[:, cc, 128 * s:128 * (s + 1)],
                            rhs=bvec_r[:, cc, :],
                            start=(cc == 0), stop=(cc == 1))
                bias_all = singles.tile([128, 6], F32, tag="bias_all")
                nc.vector.tensor_tensor(out=bias_all, in0=qkvb_sb,
                                        in1=bps[0:128, 0:12].rearrange("p (s j) -> p s j", j=2)[:, :, 0],
                                        op=ALU.add)
                # pbtot = proj_b + P @ bias_v  (folds the v bias through proj)
                bvv = singles.tile([128, 2, 2], F32R, tag="bvv")
                for ds in range(2):
                    nc.vector.tensor_copy(out=bvv[:, ds, :], in_=_bcast_free(bias_all[:, 4 + ds:5 + ds], 2))
                pbps = accps.tile([128, 512], F32, tag="acc", name="pbps")
                for mc in range(2):
                    for ds in range(2):
                        nc.tensor.matmul(
                            pbps[0:128, 2 * mc:2 * mc + 2], lhsT=projT32_sb[:, ds, 128 * mc:128 * (mc + 1)],
                            rhs=bvv[:, ds, :], start=(ds == 0), stop=(ds == 1))
                pbtot = singles.tile([128, 2], F32, tag="pbtot")
                nc.vector.tensor_tensor(out=pbtot, in0=projbrc_sb,
                                        in1=pbps[0:128, 0:4].rearrange("p (s j) -> p s j", j=2)[:, :, 0],
                                        op=ALU.add)
                # scaled weight copies: bf16 (k/v-gen vs bf16 x) and f32r
                # q-slice (q-gen vs f32r xq)
                wTs_b = singles.tile([128, 2, 2 * C], BF16, tag="wTs_b")
                wTs_q = singles.tile([128, 2, C], F32R, tag="wTs_q")
                for cc in range(2):
                    nc.vector.tensor_scalar_mul(
                        out=wTs_b[:, cc, :], in0=wT_sb[:, cc, C:3 * C],
                        scalar1=ab_sb[:, cc, 0:1])
                    nc.vector.tensor_scalar_mul(
                        out=wTs_q[:, cc, :], in0=wT_sb[:, cc, 0:C],
                        scalar1=ab_sb[:, cc, 0:1])

                # ---- fp8 k/v/q caches ----
                # k/q: [p, slice, col]; slice s holds heads (2s, 2s+1): head
                # h = 2s + (p>=64), dim d = p%64. This is exactly the partition
                # layout the k/q-gen matmuls produce, so the fp8 bias-add
                # writes the cache directly - no fold DMAs, no staging.
                k2_sb = singles.tile([128, 2, SEQ], FP8, tag="k2_sb", name="k2_sb")
                # v: [key-in-chunk, chunk, head, 72]; dims at 0:64, ones at 64
                v_sb = singles.tile([128, NCH, HEADS, 72], FP8, tag="v_sb", name="v_sb")
                ones_f = singles.tile([128, 1], F32, tag="ones_f")
                nc.vector.memset(ones_f, 1.0)
                ones_v_bc = bass.AP(tensor=ones_f.tensor, offset=ones_f.offset,
                                    ap=[list(ones_f.ap[0]), [0, NCH], [0, HEADS], [1, 1]])
                nc.vector.tensor_copy(out=v_sb[:, :, :, 64:65], in_=ones_v_bc)
                q2_sb = singles.tile([128, 2, NQC], FP8, tag="q2_sb", name="q2_sb")
                expb_sb = singles.tile([128, 1], F32, tag="expb")
                nc.vector.memset(expb_sb, EXPB)
                ones_r = singles.tile([1, 64], F32R, tag="ones_r")
                nc.vector.tensor_copy(out=ones_r, in_=_bcast_free(ones_f[0:1, 0:1], 64))

                # ---- q generation (per window, both dim-slices) ----
                def emit_q(wsel):
                    for s in range(2):  # dim slice: heads (0,1) / (2,3)
                        for w, (q0, qn, _, _) in ((w, WINDOWS[w]) for w in wsel):
                            qps = accps.tile([128, 512], F32, tag="acc", name="qps")
                            for cc in range(2):
                                nc.tensor.matmul(
                                    qps[:, 0:qn],
                                    lhsT=wTs_q[:, cc, 128 * s:128 * s + 128],
                                    rhs=xq_sb[:, cc, q0:q0 + qn], start=(cc == 0), stop=(cc == 1))
                            nc.vector.tensor_scalar_add(
                                out=q2_sb[:, s, q0:q0 + qn], in0=qps[:, 0:qn],
                                scalar1=bias_all[:, s:s + 1])

                # ---- k/v generation for one seq tile ----
                def emit_kv_tile(st):
                    s0 = ST * st
                    for s in range(2):  # k dim slice: heads (0,1) / (2,3)
                        kps = accps.tile([128, 512], F32, tag="acc", name="kps")
                        for cc in range(2):
                            nc.tensor.matmul(
                                kps, lhsT=wTs_b[:, cc, 128 * s:128 * s + 128],
                                rhs=x_sb[:, cc, s0:s0 + ST], start=(cc == 0), stop=(cc == 1))
                        nc.vector.tensor_scalar_add(
                            out=k2_sb[:, s, s0:s0 + ST], in0=kps,
                            scalar1=bias_all[:, 2 + s:3 + s])
                    # v: 4 token chunks of 128; out [128 tokens, 256 dims]
                    vps = accps.tile([128, 512], F32, tag="acc", name="vps")
                    for mc in range(4):
                        o = 256 * (mc % 2)
                        for cc in range(2):
                            nc.tensor.matmul(
                                vps[:, o:o + 256],
                                lhsT=x_sb[:, cc, s0 + 128 * mc:s0 + 128 * (mc + 1)],
                                rhs=wTs_b[:, cc, C:2 * C], start=(cc == 0), stop=(cc == 1))
                        ch = 4 * st + mc
                        vv = vps[:, o:o + 256].rearrange("p (h d) -> p h d", h=4)
                        nc.vector.tensor_copy(out=v_sb[:, ch, :, 0:64], in_=vv)

                # ---- attention stream for one (window, head) ----
                # attn layout: slice j holds heads (j, 2+j); head h -> slice h%2,
                # partitions 64*(h//2). projTb rows are host-permuted to match.
                def gen_stream(w, h):
                    q0, qn, kc0, nch = WINDOWS[w]
                    o_t = ops.tile([128, 512], F32, tag="o", name="o_t") if not PROBE else None
                    npair = nch // 2
                    if PROBE == 'noqk':
                        for pr in range(npair):
                            yield
                        dbase = 64 * (h // 2)
                        nc.vector.tensor_copy(
                            out=attn_sb[dbase:dbase + 64, h % 2, q0:q0 + qn],
                            in_=_bcast_free(ones_f[0:64, 0:1], qn))
                        return
                    pb = 64 * (h % 2)
                    sl = h // 2
                    for pr in range(npair):
                        c0 = kc0 + 2 * pr
                        s_ps = sps.tile([128, 2, 512], F32, tag="s", name="s_ps")
                        for j in range(2):
                            nc.tensor.matmul(
                                s_ps[:, j, 0:qn],
                                lhsT=k2_sb[pb:pb + 64, sl, 128 * (c0 + j):128 * (c0 + j) + 128],
                                rhs=q2_sb[pb:pb + 64, sl, q0:q0 + qn],
                                start=True, stop=True,
                                tile_position=(pb, 0))
                        if PROBE != 'noexp':
                            pt = ptp.tile([128, 2, 512], FP8, tag="p", name="pt")
                            if pr % 8 in DVE_PRS:
                                nc.vector.tensor_scalar(
                                    out=pt[:, :, 0:qn].bitcast(U8), in0=s_ps[:, :, 0:qn],
                                    scalar1=SCH_A, scalar2=SCH_B,
                                    op0=ALU.mult, op1=ALU.add)
                            else:
                                nc.scalar.activation(out=pt[:, :, 0:qn], in_=s_ps[:, :, 0:qn],
                                                     func=AF.Exp, scale=SCALE, bias=expb_sb)
                            if not PROBE:
                                for j in range(2):
                                    nc.tensor.matmul(
                                        o_t[0:65, 0:qn],
                                        lhsT=v_sb[:, c0 + j, h, 0:65],
                                        rhs=pt[:, j, 0:qn],
                                        start=(pr == 0 and j == 0),
                                        stop=(pr == npair - 1 and j == 1))
                        yield
                    if PROBE:
                        dbase = 64 * (h // 2)
                        nc.vector.tensor_copy(
                            out=attn_sb[dbase:dbase + 64, h % 2, q0:q0 + qn],
                            in_=_bcast_free(ones_f[0:64, 0:1], qn))
                        return
                    # epilogue: normalize by reciprocal of sums row 64,
                    # partition-broadcast via a rank-1 PE matmul (ones x rec)
                    osb = epi.tile([65, 512], F32, tag="osb", name="osb")
                    nc.vector.tensor_copy(out=osb[:, 0:qn], in_=o_t[0:65, 0:qn])
                    rec = epi.tile([1, 512], F32R, tag="rec", name="rec")
                    with nc.allow_low_precision(reason="f32r out is bit-identical to f32"):
                        nc.vector.reciprocal(out=rec[0:1, 0:qn], in_=osb[64:65, 0:qn])
                    recb = accps.tile([128, 512], F32, tag="acc", name="recb")
                    nc.tensor.matmul(recb[0:64, 0:qn], lhsT=ones_r, rhs=rec[0:1, 0:qn],
                                     start=True, stop=True)
                    if h < 2:
                        nc.vector.tensor_tensor(
                            out=attn_sb[0:64, h, q0:q0 + qn],
                            in0=osb[0:64, 0:qn], in1=recb[0:64, 0:qn], op=ALU.mult)
                    else:
                        at = epi.tile([64, 512], BF16, tag="at", name="at")
                        nc.vector.tensor_tensor(
                            out=at[:, 0:qn], in0=osb[0:64, 0:qn], in1=recb[0:64, 0:qn], op=ALU.mult)
                        nc.sync.dma_start(out=attn_sb[64:128, h - 2, q0:q0 + qn], in_=at[:, 0:qn])

                attn_sb = singles.tile([128, 2, NQC], BF16, tag="attn", name="attn_sb")

                def emit_proj(w):
                    q0, qn, _, _ = WINDOWS[w]
                    for mc in range(2):
                        pp = accps.tile([128, 512], F32, tag="acc", name="pp")
                        for ds in range(2):
                            nc.tensor.matmul(
                                pp[:, 0:qn], lhsT=projTb_sb[:, ds, 128 * mc:128 * (mc + 1)],
                                rhs=attn_sb[:, ds, q0:q0 + qn], start=(ds == 0), stop=(ds == 1))
                        ot = outp.tile([128, 512], F32, tag="ot", name="ot")
                        nc.vector.tensor_scalar_add(out=ot[:, 0:qn], in0=pp[:, 0:qn],
                                                    scalar1=pbtot[:, mc:mc + 1])
                        nc.vector.tensor_tensor(out=ot[:, 0:qn], in0=ot[:, 0:qn],
                                                in1=xq_sb[:, mc, q0:q0 + qn], op=ALU.add)
                        nc.sync.dma_start(out=out_d[128 * mc:128 * (mc + 1), q0:q0 + qn], in_=ot[:, 0:qn])

                # ---- schedule ----
                emit_q([0])

                # head-pair round-robin streams per window, gated on kv tiles.
                # stream (w,h) pair pr needs chunks kc0+2pr+1 <= last emitted
                def ready_pairs(w, last_chunk):
                    _, _, kc0, nch = WINDOWS[w]
                    return min(nch // 2, max(0, (last_chunk - kc0 - 1) // 2 + 1))

                # phase 1: emit kv tiles, interleaving W0 streams for heads 0,1
                # then 2,3; then W1, W2 the same way once their chunks exist.
                gens = {}
                done_pairs = {}
                for w in range(3):
                    for h in range(HEADS):
                        gens[(w, h)] = gen_stream(w, h)
                        done_pairs[(w, h)] = 0

                def advance(w, hpair, upto):
                    """Advance streams (w, 2*hpair) and (w, 2*hpair+1) round-robin
                    until each has emitted `upto` pairs (or exhausted)."""
                    hs = (2 * hpair, 2 * hpair + 1)
                    while True:
                        progressed = False
                        for h in hs:
                            if done_pairs[(w, h)] < upto:
                                try:
                                    next(gens[(w, h)])
                                    done_pairs[(w, h)] += 1
                                    progressed = True
                                except StopIteration:
                                    done_pairs[(w, h)] = 10 ** 9
                        if not progressed:
                            break

                # +1: the final next() runs the epilogue (StopIteration)
                NP = [WINDOWS[w][3] // 2 + 1 for w in range(3)]
                for st in range(NST):
                    emit_kv_tile(st)
                    if st == 2:
                        emit_q([1])
                    elif st == 4:
                        emit_q([2])
                    last_chunk = 4 * st + 3
                    # W0 heads 0,1 track the kv frontier; heads 2,3 fill the
                    # tail tiles (W0 chunks all ready from tile 11)
                    advance(0, 0, ready_pairs(0, last_chunk))
                    if st >= 11:
                        advance(0, 0, NP[0])  # epilogues before heads 2,3 start
                    if st >= 12:
                        advance(0, 1, 8 * (st - 11))
                advance(0, 0, NP[0])
                advance(0, 1, NP[0])
                # start the next window before emitting proj so PE doesn't
                # stall on the previous window's epilogue chain
                advance(1, 0, 2)
                emit_proj(0)
                advance(1, 0, NP[1])
                advance(1, 1, NP[1])
                advance(2, 1, 2)
                emit_proj(1)
                advance(2, 1, NP[2])
                advance(2, 0, NP[2])
                emit_proj(2)

    return nc


def make_inputs(x, norm_w, norm_b, qkv_w, qkv_b, proj_w, proj_b):
    """Host-side prep: full-input numpy -> per-core in_maps."""
    x2 = np.ascontiguousarray(np.asarray(x, np.float32).reshape(C, SEQ))
    qkv_w = np.asarray(qkv_w, np.float32)
    qkv_b = np.asarray(qkv_b, np.float32)
    proj_w = np.asarray(proj_w, np.float32)
    proj_b = np.asarray(proj_b, np.float32)
    norm_w = np.asarray(norm_w, np.float32)
    norm_b = np.asarray(norm_b, np.float32)

    wT = np.ascontiguousarray(qkv_w.T)
    projT32 = np.ascontiguousarray(proj_w.T)
    # attn slice j holds heads (j, 2+j): permute projT rows to match
    perm = np.array([64 * (j + 2 * (p >= 64)) + (p % 64)
                     for j in range(2) for p in range(128)])
    projTb = np.ascontiguousarray(projT32[perm, :]).astype(ml_dtypes.bfloat16)
    qkvb = np.ascontiguousarray(qkv_b.reshape(6, 128).T)
    nwb = np.stack([norm_w[0:128], norm_w[128:256], norm_b[0:128], norm_b[128:256]], axis=1)
    projbrc = np.ascontiguousarray(proj_b.reshape(2, 128).T)
    cidx = np.arange(128)
    gidx = np.arange(16)
    G = ((cidx[:, None] // 8) == gidx[None, :]).astype(np.float32) / 8.0
    GT = np.ascontiguousarray(G.T * 8.0)

    xb = x2.astype(ml_dtypes.bfloat16)
    common = dict(x=xb, wT=wT, projTb=projTb, projT32=projT32, qkvb=qkvb,
                  nwb=nwb, projbrc=projbrc, G=G, GT=GT)
    in_maps = []
    cols = []
    for i in range(NCORES):
        ci = np.concatenate([
            np.arange(512 * i, 512 * (i + 1)),
            np.arange(4096 + 512 * i, 4096 + 512 * (i + 1)),
            np.arange(8192 + 128 * i, 8192 + 128 * (i + 1)),
        ])
        cols.append(ci)
        m = dict(common)
        m["xq"] = np.ascontiguousarray(x2[:, ci])
        in_maps.append(m)
    return in_maps, cols


_NC_CACHE = {}


def kernel(x, norm_w, norm_b, qkv_w, qkv_b, proj_w, proj_b):
    from concourse.bass_utils import run_bass_kernel_spmd

    _patch_tile_drain()
    _patch_to_json_split_waits()
    in_maps, cols = make_inputs(x, norm_w, norm_b, qkv_w, qkv_b, proj_w, proj_b)
    if "nc" not in _NC_CACHE:
        _NC_CACHE["nc"] = build_nc()
    nc = _NC_CACHE["nc"]
    res = run_bass_kernel_spmd(nc, in_maps, core_ids=list(range(NCORES)))
    out = np.zeros((C, SEQ), np.float32)
    for i in range(NCORES):
        out[:, cols[i]] = res.results[i]["out"]
    return out.reshape(1, C, 96, 96)

